# revision 31
# baseline (speedup 1.0000x reference)
"""Bidirectional Mamba (PartContextMamba) Trainium2 Bass kernel, v2.

Sharding: pure data parallelism over batch (1024 -> 8 cores x 128 batch).
Token order on all free axes is (l, b) -- l OUTER, b inner (host reshapes
x to l-major). This makes every scan-phase access contiguous:

  xT [768d, (l b)] f16 (PE transpose of the x shard)
  per direction (fwd, bwd):
    xi = W_in_xi @ xT (PE), conv via shifted-slab STTs (DVE), silu (ACT)
    x_dbl = W_xp @ xc -> dt_lo[48] f32, B[16], C[16] f16
    B/C staged to DRAM as (half, t, n, b) then partition-broadcast to all
    128 partitions (brep/crep).
    per mt (12 d-tiles of 128):
      dt = softplus(W_dt @ dt_lo + dt_b)  (ACT Exp+Ln, one table)
      wt = dt*xc (DVE)
      per b-half h (64):
        pow[n,t,b] = exp(-(n+1)dt): 8 ACT exps + 1 DVE doubling TT
        wbh[t,n,b] = wt x brep (DVE TT, becomes h in place)
        recurrence h[t] = pow[t]*h[t-1] + wbh[t]: 10 unrolled TTs
        (in-place over wbh; bwd runs the slab loop in reverse)
        hc = h*crep -> pow buffer; log-tree reduce over n (GpSimd TTs,
        ping-pong between wbh/pow buffers) -> yfin f32
        y = yfin + D*xc (STT) -> ygated slot
    z-gate: ygated *= silu(W_z @ xT) (PE + ACT Silu + DVE TT)
    yout += W_out @ ygated (PE, PSUM k-accum)
  out = LayerNorm(x + yout^T) (PE transpose, ACT Rsqrt, token-major)
"""

import numpy as np

_CACHE: dict = {}

B = 128          # batch per core
L = 6
D = 768
DI = 1536
NT = 12          # d-tiles
NS = 16          # ssm states
R = 48           # dt rank
TOK = B * L
ET = 6           # token-tiles (now l-slabs)
KT = 6           # k-tiles of D
NH = 2           # b-halves
BH = B // NH     # 64
HV = NS * BH * L  # 6144 elems per half of brep/crep
SL = NS * BH     # 1024, one t-slab in (t,n,b)


def _build_module(debug=False):
    import concourse.bass as bass
    import concourse.bacc as bacc
    import concourse.mybir as mybir
    import concourse.tile as tile
    from concourse.masks import make_identity

    f32 = mybir.dt.float32
    f16 = mybir.dt.float16
    AP = bass.AP
    AF = mybir.ActivationFunctionType
    OP = mybir.AluOpType

    nc = bacc.Bacc("TRN2", target_bir_lowering=False)

    x_d = nc.dram_tensor("x", [TOK, D], f32, kind="ExternalInput")
    ins = {}
    for d in ("f", "b"):
        ins[f"win_{d}"] = nc.dram_tensor(f"win_{d}", [D, DI], f16, kind="ExternalInput")
        ins[f"wz_{d}"] = nc.dram_tensor(f"wz_{d}", [NT, 128, KT, 128], f16, kind="ExternalInput")
        ins[f"wxp_{d}"] = nc.dram_tensor(f"wxp_{d}", [128, NT, 80], f16, kind="ExternalInput")
        ins[f"wdt_{d}"] = nc.dram_tensor(f"wdt_{d}", [R, DI], f16, kind="ExternalInput")
        ins[f"wout_{d}"] = nc.dram_tensor(f"wout_{d}", [DI, D], f16, kind="ExternalInput")
        ins[f"aux_{d}"] = nc.dram_tensor(f"aux_{d}", [DI, 8], f32, kind="ExternalInput")
        ins[f"cwd_{d}"] = nc.dram_tensor(f"cwd_{d}", [NT, 128, 4, 128], f16, kind="ExternalInput")
    lng_d = nc.dram_tensor("ln_g", [D], f32, kind="ExternalInput")
    lnb_d = nc.dram_tensor("ln_b", [D], f32, kind="ExternalInput")
    out_d = nc.dram_tensor("out", [TOK, D], f32, kind="ExternalOutput")

    def dram_ap(t, offset, ap):
        return AP(tensor=t, offset=offset, ap=ap)

    def dbg(name, ap):
        if not debug:
            return
        p = ap.partition_size()
        counts = [c for _, c in ap.ap[1:]]
        t = nc.dram_tensor(f"dbg_{name}", [p] + counts, ap.dtype,
                           kind="ExternalOutput")
        nc.sync.dma_start(t[:], ap)

    with tile.TileContext(nc) as tc:
        with (
            tc.tile_pool(name="consts", bufs=1) as consts,
            tc.tile_pool(name="persist", bufs=1) as persist,
            tc.tile_pool(name="wpool", bufs=1) as wpool,
            tc.tile_pool(name="wstream", bufs=2) as wstream,
            tc.tile_pool(name="tr2", bufs=2) as tr2,
            tc.tile_pool(name="tr1", bufs=1) as tr1,
            tc.tile_pool(name="scanp", bufs=2) as scanp,
            tc.tile_pool(name="reps", bufs=1) as repsp,
            tc.tile_pool(name="dram", bufs=1, space="DRAM") as dramp,
            tc.tile_pool(name="psA", bufs=2, space="PSUM") as psA,
            tc.tile_pool(name="psT", bufs=1, space="PSUM") as psT,
            tc.tile_pool(name="psO", bufs=1, space="PSUM") as psO,
        ):
            # ---------------- constants ----------------
            ident = consts.tile([128, 128], f32)
            make_identity(nc, ident)
            identh = consts.tile([128, 128], f16)
            nc.vector.tensor_copy(identh[:], ident[:])
            g_rep = consts.tile([128, D], f32)
            nc.sync.dma_start(g_rep[:], dram_ap(lng_d, 0, [[0, 128], [1, D]]))
            b_rep = consts.tile([128, D], f32)
            nc.sync.dma_start(b_rep[:], dram_ap(lnb_d, 0, [[0, 128], [1, D]]))
            eps_t = consts.tile([128, 1], f32)
            nc.vector.memset(eps_t[:], 1e-5)
            aux = {}
            for d in ("f", "b"):
                aux[d] = consts.tile([128, NT, 8], f32, tag=f"aux_{d}", name=f"aux_{d}")
                nc.sync.dma_start(
                    aux[d][:],
                    dram_ap(ins[f"aux_{d}"], 0, [[8, 128], [8 * 128, NT], [1, 8]]),
                )

            # ---------------- xT (fp16) via PE transpose ----------------
            # x_d rows are tokens in (l, b) order (host reshaped l-major),
            # so chunk tt == l-slab tt.
            xT = persist.tile([128, KT, TOK], f16, tag="xT")
            for tt in range(ET):
                xtok = tr1.tile([128, D], f32, tag="xtok")
                nc.sync.dma_start(xtok[:], x_d[tt * 128:(tt + 1) * 128, :])
                for ec in range(KT):
                    pst = psT.tile([128, 128], f32, tag="pst")
                    nc.tensor.transpose(pst[:], xtok[:, ec * 128:(ec + 1) * 128], ident[:])
                    nc.scalar.copy(xT[:, ec, tt * 128:(tt + 1) * 128], pst[:])

            dbg("xT", xT[:])
            yout = persist.tile([128, ET, TOK], f16, tag="yout")
            xc = persist.tile([128, NT, TOK], f16, tag="xc")

            ygz_d = {
                d: dramp.tile([NT, 128, TOK], f16, tag=f"ygz_{d}", name=f"ygz_{d}")
                for d in ("f", "b")
            }

            def phase1_mt(d, mt):
                fwd = d == "f"
                if True:
                    win_t = wstream.tile([128, KT, 128], f16, tag="win_t")
                    for kt in range(KT):
                        nc.sync.dma_start(
                            win_t[:, kt, :],
                            dram_ap(ins[f"win_{d}"],
                                    kt * 128 * DI + mt * 128,
                                    [[DI, 128], [1, 128]]),
                        )
                    ps = psA.tile([128, 2, 512], f32, tag="psA")
                    for kt in range(KT):
                        for ng in range(2):
                            nc.tensor.matmul(
                                ps[:, ng, 0:384],
                                win_t[:, kt, :],
                                xT[:, kt, ng * 384:(ng + 1) * 384],
                                start=(kt == 0),
                                stop=(kt == KT - 1),
                            )
                    xi_t = tr2.tile([128, TOK], f16, tag="scr16")
                    for ng in range(2):
                        nc.scalar.copy(xi_t[:, ng * 384:(ng + 1) * 384],
                                       ps[:, ng, 0:384])

                    # conv via PE: diag(cw_k) matmuls, PSUM-accumulated.
                    # fwd: xc[l] = sum_k w[k]*xi[l+k-3]; bwd: sum_j w[3-j]*xi[l+j]
                    wcv = wstream.tile([128, 4, 128], f16, tag="wcv")
                    nc.sync.dma_start(wcv[:], ins[f"cwd_{d}"][mt, :, :, :])
                    ps2 = psA.tile([128, 2, 512], f32, tag="psA")
                    if fwd:
                        taps = {
                            0: [(3, 0, 384, 0), (2, 128, 384, 0), (1, 256, 384, 0)],
                            1: [(3, 0, 384, 384), (2, 0, 384, 256),
                                (1, 0, 384, 128), (0, 0, 384, 0)],
                        }
                    else:
                        taps = {
                            0: [(3, 0, 384, 0), (2, 0, 384, 128),
                                (1, 0, 384, 256), (0, 0, 384, 384)],
                            1: [(3, 0, 384, 384), (2, 0, 256, 512),
                                (1, 0, 128, 640)],
                        }
                    for ng, tl in taps.items():
                        for i, (k, o0, o1, xo) in enumerate(tl):
                            nc.tensor.matmul(
                                ps2[:, ng, o0:o1],
                                wcv[:, k, :],
                                xi_t[:, xo:xo + (o1 - o0)],
                                start=(i == 0), stop=(i == len(tl) - 1),
                            )
                    nc.scalar.activation(
                        out=xc[:, mt, :],
                        in_=ps2[:, :, 0:384],
                        func=AF.Silu,
                        bias=aux[d][:, mt, 4:5],
                    )

            def phase2(d):
                dbg(f"xc_{d}", xc[:])
                # ---------------- phase 2: x_proj -> dt_lo, B, C -----------
                wxp = wpool.tile([128, NT, 80], f16, tag="wxp")
                nc.sync.dma_start(wxp[:], ins[f"wxp_{d}"][:])
                dt_lo = tr1.tile([R, TOK], f16, tag="dt_lo")
                bc_sb = tr1.tile([16, 2, TOK], f16, tag="bc_sb")
                for part, (m0, m1) in enumerate([(0, 48), (48, 64), (64, 80)]):
                    psx = psA.tile([128, 2, 512], f32, tag="psA")
                    for kt in range(NT):
                        for ng in range(2):
                            nc.tensor.matmul(
                                psx[:m1 - m0, ng, 0:384],
                                wxp[:, kt, m0:m1],
                                xc[:, kt, ng * 384:(ng + 1) * 384],
                                start=(kt == 0),
                                stop=(kt == NT - 1),
                            )
                    for ng in range(2):
                        if part == 0:
                            nc.scalar.copy(dt_lo[:, ng * 384:(ng + 1) * 384],
                                           psx[:R, ng, 0:384])
                        else:
                            nc.scalar.copy(
                                bc_sb[:, part - 1, ng * 384:(ng + 1) * 384],
                                psx[:16, ng, 0:384],
                            )

                # stage B/C to DRAM as (half, t, n, b) then broadcast-read
                bstage = dramp.tile([NH, L, NS, BH], f16, tag="bstage")
                cstage = dramp.tile([NH, L, NS, BH], f16, tag="cstage")
                for part, stg in ((0, bstage), (1, cstage)):
                    for h in range(NH):
                        for t in range(L):
                            nc.sync.dma_start(
                                stg[h, t, :, :],
                                bc_sb[:, part, t * 128 + h * BH:
                                      t * 128 + h * BH + BH],
                            )
                brep = repsp.tile([128, NH, L, NS, BH], f16, tag="brep")
                sa = bstage[:, :, :, :]
                nc.sync.dma_start(
                    brep[:],
                    AP(tensor=sa.tensor, offset=sa.offset,
                       ap=[[0, 128], [1, NH * HV]]),
                )
                crep = repsp.tile([128, NH, L, NS, BH], f16, tag="crep")
                sc = cstage[:, :, :, :]
                nc.sync.dma_start(
                    crep[:],
                    AP(tensor=sc.tensor, offset=sc.offset,
                       ap=[[0, 128], [1, NH * HV]]),
                )

                dbg(f"dtlo_{d}", dt_lo[:])
                dbg(f"bc_{d}", bc_sb[:])
                st["dt_lo"] = dt_lo
                st["brep"] = brep
                st["crep"] = crep

            st = {}
            ygated = persist.tile([128, NT, TOK], f16, tag="ygated")

            def phase3(d, after_mt=None):
                """Scan + fused z-gate per mt; dir f DMAs ygated to DRAM."""
                fwd = d == "f"
                dt_lo, brep, crep = st["dt_lo"], st["brep"], st["crep"]
                for mt in range(NT):
                    # dt = softplus(wdt.T @ dt_lo + dt_b), all tokens
                    wdt_t = wstream.tile([R, 128], f16, tag="wdt_t")
                    nc.sync.dma_start(
                        wdt_t[:],
                        dram_ap(ins[f"wdt_{d}"], mt * 128, [[DI, R], [1, 128]]))
                    psd = psA.tile([128, 2, 512], f32, tag="psA")
                    for ng in range(2):
                        nc.tensor.matmul(
                            psd[:, ng, 0:384], wdt_t[:],
                            dt_lo[:, ng * 384:(ng + 1) * 384],
                            start=True, stop=True,
                        )
                    dt16 = tr2.tile([128, TOK], f16, tag="dt16")
                    for ng in range(2):
                        dte = tr2.tile([128, 384], f32, tag="dte")
                        nc.scalar.activation(
                            out=dte[:],
                            in_=psd[:, ng, 0:384], func=AF.Exp,
                            bias=aux[d][:, mt, 5:6],
                        )
                        nc.scalar.activation(
                            out=dt16[:, ng * 384:(ng + 1) * 384],
                            in_=dte[:], func=AF.Ln, bias=1.0,
                        )
                    if mt == 0:
                        dbg(f"dt_{d}", dt16[:])
                    # z-proj for the gate (PE + Silu), independent of scan
                    wz_t = wstream.tile([128, KT, 128], f16, tag="win_t")
                    nc.sync.dma_start(wz_t[:], ins[f"wz_{d}"][mt, :, :, :])
                    psz = psA.tile([128, 2, 512], f32, tag="psA")
                    for kt in range(KT):
                        for ng in range(2):
                            nc.tensor.matmul(
                                psz[:, ng, 0:384], wz_t[:, kt, :],
                                xT[:, kt, ng * 384:(ng + 1) * 384],
                                start=(kt == 0), stop=(kt == KT - 1),
                            )
                    sz_t = tr2.tile([128, TOK], f16, tag="scr16")
                    for ng in range(2):
                        nc.scalar.activation(
                            out=sz_t[:, ng * 384:(ng + 1) * 384],
                            in_=psz[:, ng, 0:384], func=AF.Silu)
                    # wt = dt * xc, (l,b) f16
                    wt_t = tr2.tile([128, TOK], f16, tag="wt_t")
                    nc.vector.tensor_tensor(
                        out=wt_t[:], in0=dt16[:], in1=xc[:, mt, :], op=OP.mult,
                    )

                    powts, wbhs, tmp6s = [], [], []
                    for h in range(NH):
                        # pow[n, (t b)] = exp(-(n+1)*dt) for this b-half
                        powt = scanp.tile([128, NS, L, BH], f16, tag="powt")
                        dt_h = AP(tensor=dt16[:].tensor,
                                  offset=dt16[:].offset + h * BH,
                                  ap=[dt16[:].ap[0], [128, L], [1, BH]])
                        for n in range(8):
                            nc.scalar.activation(
                                out=powt[:, n, :, :], in_=dt_h,
                                func=AF.Exp, scale=-(float(n + 1)),
                            )
                        p8 = powt[:, 7, :, :]
                        nc.vector.tensor_tensor(
                            out=powt[:, 8:16, :, :],
                            in0=AP(tensor=p8.tensor, offset=p8.offset,
                                   ap=[p8.ap[0], [0, 8], [1, L * BH]]),
                            in1=powt[:, 0:8, :, :],
                            op=OP.mult,
                        )
                        if mt == 0:
                            dbg(f"pow_{d}_h{h}", powt[:])
                        # wbh[t, n, b] = wt x brep; becomes h in place.
                        wbh = scanp.tile([128, L, NS, BH], f16, tag="wbh")
                        wt_v = wt_t[:]
                        nc.vector.tensor_tensor(
                            out=wbh[:],
                            in0=AP(tensor=wt_v.tensor,
                                   offset=wt_v.offset + h * BH,
                                   ap=[wt_v.ap[0], [128, L], [0, NS], [1, BH]]),
                            in1=brep[:, h, :, :, :],
                            op=OP.mult,
                        )
                        if mt == 0:
                            dbg(f"wb_{d}_h{h}", wbh[:])
                        powts.append(powt)
                        wbhs.append(wbh)
                        tmp6_h = scanp.tile([128, L, 8, BH], f16, tag="tmp6",
                                            name=f"tmp6_{h}")
                        tmp6s.append(tmp6_h)

                    # recurrence, in place: h[t] = pow[t]*h[prev] + wbh[t]
                    # halves interleaved to fill the dependent-chain bubbles
                    steps = range(1, L) if fwd else range(L - 2, -1, -1)
                    for t in steps:
                        tprev = t - 1 if fwd else t + 1
                        tslots = []
                        for h in range(NH):
                            pv = powts[h][:]
                            pow_t = AP(
                                tensor=pv.tensor, offset=pv.offset + t * BH,
                                ap=[pv.ap[0], [L * BH, NS], [1, BH]])
                            tmpf = tmp6s[h][:].rearrange("p t n b -> p (t n b)")
                            tslot = AP(tensor=tmpf.tensor, offset=tmpf.offset,
                                       ap=[tmpf.ap[0], [1, SL]])
                            tslots.append(tslot)
                            nc.vector.tensor_tensor(
                                out=tslot, in0=pow_t,
                                in1=wbhs[h][:, tprev, :, :].rearrange(
                                    "p n b -> p (n b)"),
                                op=OP.mult,
                            )
                        for h in range(NH):
                            wslab = wbhs[h][:, t, :, :].rearrange(
                                "p n b -> p (n b)")
                            nc.vector.tensor_tensor(
                                out=wslab, in0=tslots[h], in1=wslab, op=OP.add,
                            )
                    if mt == 0:
                        dbg(f"h_{d}_h0", wbhs[0][:])

                    # hc = h * crep in place (DVE); tree L1 on GpSimd
                    wbfs = [wbhs[h][:].rearrange("p t n b -> p (t n b)")
                            for h in range(NH)]
                    tmpfs = [tmp6s[h][:].rearrange("p t n b -> p (t n b)")
                             for h in range(NH)]
                    HSL = 8 * BH

                    def lvl(eng, src, sw, dst, dw, n_out):
                        w = n_out * BH
                        i0 = AP(tensor=src.tensor, offset=src.offset,
                                ap=[src.ap[0], [sw, L], [1, w]])
                        i1 = AP(tensor=src.tensor, offset=src.offset + w,
                                ap=[src.ap[0], [sw, L], [1, w]])
                        o = AP(tensor=dst.tensor, offset=dst.offset,
                               ap=[dst.ap[0], [dw, L], [1, w]])
                        eng.tensor_tensor(out=o, in0=i0, in1=i1, op=OP.add)

                    for h in range(NH):
                        nc.vector.tensor_tensor(
                            out=wbfs[h], in0=wbfs[h],
                            in1=crep[:, h, :, :, :].rearrange(
                                "p t n b -> p (t n b)"),
                            op=OP.mult,
                        )
                        lvl(nc.gpsimd if h == 0 else nc.vector,
                            wbfs[h], SL, tmpfs[h], HSL, 8)
                    # gp chains L2(h0); DVE finishes h1's tree meanwhile
                    lvl(nc.gpsimd, tmpfs[0], HSL, wbfs[0], SL, 4)
                    yfins = [None, None]
                    for h in (1, 0):
                        if h == 1:
                            lvl(nc.vector, tmpfs[1], HSL, wbfs[1], SL, 4)
                        lvl(nc.vector, wbfs[h], SL, tmpfs[h], HSL, 2)
                        yfin = scanp.tile([128, L, BH], f16, tag="yfin",
                                          name=f"yfin{h}")
                        yfins[h] = yfin
                        nc.vector.tensor_tensor(
                            out=yfin[:].rearrange("p t b -> p (t b)"),
                            in0=AP(tensor=tmpfs[h].tensor,
                                   offset=tmpfs[h].offset,
                                   ap=[tmpfs[h].ap[0], [HSL, L], [1, BH]]),
                            in1=AP(tensor=tmpfs[h].tensor,
                                   offset=tmpfs[h].offset + BH,
                                   ap=[tmpfs[h].ap[0], [HSL, L], [1, BH]]),
                            op=OP.add,
                        )
                        if mt == 0:
                            dbg(f"yfin_{d}_h{h}", yfin[:])
                    # skip term: ygated slot = D*xc + yfin, then z-gate
                    og = ygated[:, mt, :]
                    for h in range(NH):
                        nc.vector.scalar_tensor_tensor(
                            out=AP(tensor=og.tensor, offset=og.offset + h * BH,
                                   ap=[og.ap[0], [128, L], [1, BH]]),
                            in0=AP(tensor=xc[:, mt, :].tensor,
                                   offset=xc[:, mt, :].offset + h * BH,
                                   ap=[og.ap[0], [128, L], [1, BH]]),
                            scalar=aux[d][:, mt, 6:7],
                            in1=yfins[h][:].rearrange("p t b -> p (t b)"),
                            op0=OP.mult, op1=OP.add,
                        )
                    nc.vector.tensor_tensor(out=og, in0=og, in1=sz_t[:],
                                            op=OP.mult)
                    if d == "f":
                        nc.sync.dma_start(ygz_d[d][mt, :, :], og)
                    if after_mt and mt in after_mt:
                        after_mt[mt]()
                dbg(f"ygated_{d}", ygated[:])

            def phase4_group(d, dir_i, ng, mg, from_dram):
                pso = psO.tile([128, 3, 512], f32, tag="psO")
                for kt in range(NT):
                    wo_t = wstream.tile([128, 3, 128], f16, tag="wo_t")
                    nc.sync.dma_start(
                        wo_t[:],
                        dram_ap(
                            ins[f"wout_{d}"],
                            kt * 128 * D + mg * 384,
                            [[D, 128], [128, 3], [1, 128]],
                        ),
                    )
                    if from_dram:
                        yg_t = wstream.tile([128, 384], f16, tag="yg_t")
                        ygv = ygz_d[d][:, :, :]
                        nc.sync.dma_start(
                            yg_t[:],
                            AP(tensor=ygv.tensor,
                               offset=ygv.offset + kt * 128 * TOK + ng * 384,
                               ap=[[TOK, 128], [1, 384]]),
                        )
                        rhs = yg_t[:]
                    else:
                        rhs = ygated[:, kt, ng * 384:(ng + 1) * 384]
                    for m in range(3):
                        nc.tensor.matmul(
                            pso[:, m, 0:384], wo_t[:, m, :], rhs,
                            start=(kt == 0), stop=(kt == NT - 1),
                        )
                for m in range(3):
                    mt_e = mg * 3 + m
                    o = yout[:, mt_e, ng * 384:(ng + 1) * 384]
                    if dir_i == 0:
                        nc.scalar.copy(o, pso[:, m, 0:384])
                    else:
                        nc.vector.tensor_tensor(
                            out=o, in0=o, in1=pso[:, m, 0:384], op=OP.add)

            # ---- pipelined emission order ----
            for mt in range(NT):
                phase1_mt("f", mt)
            phase2("f")
            ph1b = {mt: (lambda m=mt: phase1_mt("b", m)) for mt in range(NT)}
            phase3("f", after_mt=ph1b)
            phase2("b")
            ph4f = {
                2: lambda: phase4_group("f", 0, 0, 0, True),
                5: lambda: phase4_group("f", 0, 0, 1, True),
                8: lambda: phase4_group("f", 0, 1, 0, True),
                11: lambda: phase4_group("f", 0, 1, 1, True),
            }
            phase3("b", after_mt=ph4f)
            for ng in range(2):
                for mg in range(2):
                    phase4_group("b", 1, ng, mg, False)

            dbg("yout", yout[:])
            # ---------------- phase 5: residual + LN -> out --------------
            for tt in range(ET):
                xtok = tr1.tile([128, D], f32, tag="xtok")
                nc.sync.dma_start(xtok[:], x_d[tt * 128:(tt + 1) * 128, :])
                r_t = tr1.tile([128, D], f32, tag="r_t")
                for ec in range(KT):
                    psh = psT.tile([128, 128], f16, tag="pst")
                    nc.tensor.transpose(
                        psh[:], yout[:, ec, tt * 128:(tt + 1) * 128], identh[:]
                    )
                    nc.vector.tensor_tensor(
                        out=r_t[:, ec * 128:(ec + 1) * 128],
                        in0=psh[:], in1=xtok[:, ec * 128:(ec + 1) * 128],
                        op=OP.add,
                    )
                stats = tr1.tile([128, 3, nc.vector.BN_STATS_DIM], f32, tag="stats")
                for sub in range(3):
                    nc.vector.bn_stats(
                        out=stats[:, sub, :], in_=r_t[:, sub * 256:(sub + 1) * 256]
                    )
                mv = tr1.tile([128, nc.vector.BN_AGGR_DIM], f32, tag="mv")
                nc.vector.bn_aggr(out=mv[:], in_=stats[:])
                rstd = tr1.tile([128, 1], f32, tag="rstd")
                nc.scalar.activation(
                    out=rstd[:], in_=mv[:, 1:2], func=AF.Sqrt, bias=eps_t[:],
                )
                nc.vector.reciprocal(out=rstd[:], in_=rstd[:])
                nc.vector.tensor_scalar(
                    out=r_t[:], in0=r_t[:], scalar1=mv[:, 0:1], scalar2=rstd[:],
                    op0=OP.subtract, op1=OP.mult,
                )
                nc.vector.tensor_tensor(out=r_t[:], in0=r_t[:], in1=g_rep[:], op=OP.mult)
                nc.vector.tensor_tensor(out=r_t[:], in0=r_t[:], in1=b_rep[:], op=OP.add)
                nc.sync.dma_start(out_d[tt * 128:(tt + 1) * 128, :], r_t[:])

    nc.compile()
    return nc


def _prep_inputs(inputs):
    f16 = np.float16
    shared = {}
    for d in ("f", "b"):
        in_proj = np.asarray(inputs[f"{d}_in"], np.float32)      # [3072, 768]
        shared[f"win_{d}"] = np.ascontiguousarray(in_proj[:DI].T).astype(f16)
        wz_T = in_proj[DI:].T                                    # [768, 1536]
        shared[f"wz_{d}"] = np.ascontiguousarray(
            wz_T.reshape(KT, 128, NT, 128).transpose(2, 1, 0, 3)
        ).astype(f16)
        xp_T = np.asarray(inputs[f"{d}_xp"], np.float32).T       # [1536, 80]
        shared[f"wxp_{d}"] = np.ascontiguousarray(
            xp_T.reshape(NT, 128, 80).transpose(1, 0, 2)
        ).astype(f16)
        shared[f"wdt_{d}"] = np.ascontiguousarray(
            np.asarray(inputs[f"{d}_dtw"], np.float32).T
        ).astype(f16)                                            # [48, 1536]
        shared[f"wout_{d}"] = np.ascontiguousarray(
            np.asarray(inputs[f"{d}_out"], np.float32).T
        ).astype(f16)                                            # [1536, 768]
        aux = np.zeros((DI, 8), np.float32)
        aux[:, 0:4] = np.asarray(inputs[f"{d}_cw"], np.float32).T
        aux[:, 4] = np.asarray(inputs[f"{d}_cb"], np.float32)
        aux[:, 5] = np.asarray(inputs[f"{d}_dtb"], np.float32)
        aux[:, 6] = np.asarray(inputs[f"{d}_D"], np.float32)
        shared[f"aux_{d}"] = aux
        cw = np.asarray(inputs[f"{d}_cw"], np.float32)           # [4, DI]
        cwd = np.zeros((NT, 128, 4, 128), np.float32)
        idx = np.arange(128)
        for mt in range(NT):
            for k in range(4):
                cwd[mt, idx, k, idx] = cw[k, mt * 128:(mt + 1) * 128]
        shared[f"cwd_{d}"] = cwd.astype(f16)
    shared["ln_g"] = np.ascontiguousarray(np.asarray(inputs["ln_g"], np.float32))
    shared["ln_b"] = np.ascontiguousarray(np.asarray(inputs["ln_b"], np.float32))
    return shared


def kernel(**inputs):
    from concourse import bass_utils

    if "nc" not in _CACHE:
        _CACHE["nc"] = _build_module()
    nc = _CACHE["nc"]

    shared = _prep_inputs(inputs)
    x = np.asarray(inputs["x"], np.float32)
    n_cores = 8
    bs = x.shape[0] // n_cores

    in_maps = []
    for c in range(n_cores):
        m = dict(shared)
        # l-major token order: row t*B + b
        m["x"] = np.ascontiguousarray(
            x[c * bs:(c + 1) * bs].transpose(1, 0, 2).reshape(TOK, D)
        ).astype(np.float32)
        in_maps.append(m)

    res = bass_utils.run_bass_kernel_spmd(nc, in_maps, core_ids=list(range(n_cores)))
    out = np.concatenate(
        [r["out"].reshape(L, bs, D).transpose(1, 0, 2) for r in res.results],
        axis=0,
    )
    return out.astype(np.float32)


# revision 32
# speedup vs baseline: 1.0367x; 1.0367x over previous
"""Bidirectional Mamba (PartContextMamba) Trainium2 Bass kernel, v2.

Sharding: pure data parallelism over batch (1024 -> 8 cores x 128 batch).
Token order on all free axes is (l, b) -- l OUTER, b inner (host reshapes
x to l-major). This makes every scan-phase access contiguous:

  xT [768d, (l b)] f16 (PE transpose of the x shard)
  per direction (fwd, bwd):
    xi = W_in_xi @ xT (PE), conv via shifted-slab STTs (DVE), silu (ACT)
    x_dbl = W_xp @ xc -> dt_lo[48] f32, B[16], C[16] f16
    B/C staged to DRAM as (half, t, n, b) then partition-broadcast to all
    128 partitions (brep/crep).
    per mt (12 d-tiles of 128):
      dt = softplus(W_dt @ dt_lo + dt_b)  (ACT Exp+Ln, one table)
      wt = dt*xc (DVE)
      per b-half h (64):
        pow[n,t,b] = exp(-(n+1)dt): 8 ACT exps + 1 DVE doubling TT
        wbh[t,n,b] = wt x brep (DVE TT, becomes h in place)
        recurrence h[t] = pow[t]*h[t-1] + wbh[t]: 10 unrolled TTs
        (in-place over wbh; bwd runs the slab loop in reverse)
        hc = h*crep -> pow buffer; log-tree reduce over n (GpSimd TTs,
        ping-pong between wbh/pow buffers) -> yfin f32
        y = yfin + D*xc (STT) -> ygated slot
    z-gate: ygated *= silu(W_z @ xT) (PE + ACT Silu + DVE TT)
    yout += W_out @ ygated (PE, PSUM k-accum)
  out = LayerNorm(x + yout^T) (PE transpose, ACT Rsqrt, token-major)
"""

import numpy as np

_CACHE: dict = {}

B = 128          # batch per core
L = 6
D = 768
DI = 1536
NT = 12          # d-tiles
NS = 16          # ssm states
R = 48           # dt rank
TOK = B * L
ET = 6           # token-tiles (now l-slabs)
KT = 6           # k-tiles of D
NH = 2           # b-halves
BH = B // NH     # 64
HV = NS * BH * L  # 6144 elems per half of brep/crep
SL = NS * BH     # 1024, one t-slab in (t,n,b)


def _build_module(debug=False):
    import concourse.bass as bass
    import concourse.bacc as bacc
    import concourse.mybir as mybir
    import concourse.tile as tile
    from concourse.masks import make_identity

    f32 = mybir.dt.float32
    f16 = mybir.dt.float16
    AP = bass.AP
    AF = mybir.ActivationFunctionType
    OP = mybir.AluOpType

    nc = bacc.Bacc("TRN2", target_bir_lowering=False)

    x_d = nc.dram_tensor("x", [TOK, D], f32, kind="ExternalInput")
    ins = {}
    for d in ("f", "b"):
        ins[f"win_{d}"] = nc.dram_tensor(f"win_{d}", [D, DI], f16, kind="ExternalInput")
        ins[f"wz_{d}"] = nc.dram_tensor(f"wz_{d}", [NT, 128, KT, 128], f16, kind="ExternalInput")
        ins[f"wxp_{d}"] = nc.dram_tensor(f"wxp_{d}", [128, NT, 80], f16, kind="ExternalInput")
        ins[f"wdt_{d}"] = nc.dram_tensor(f"wdt_{d}", [R, DI], f16, kind="ExternalInput")
        ins[f"wout_{d}"] = nc.dram_tensor(f"wout_{d}", [DI, D], f16, kind="ExternalInput")
        ins[f"aux_{d}"] = nc.dram_tensor(f"aux_{d}", [DI, 8], f32, kind="ExternalInput")
        ins[f"cwd_{d}"] = nc.dram_tensor(f"cwd_{d}", [NT, 128, 4, 128], f16, kind="ExternalInput")
    lng_d = nc.dram_tensor("ln_g", [D], f32, kind="ExternalInput")
    lnb_d = nc.dram_tensor("ln_b", [D], f32, kind="ExternalInput")
    out_d = nc.dram_tensor("out", [TOK, D], f32, kind="ExternalOutput")

    def dram_ap(t, offset, ap):
        return AP(tensor=t, offset=offset, ap=ap)

    def dbg(name, ap):
        if not debug:
            return
        p = ap.partition_size()
        counts = [c for _, c in ap.ap[1:]]
        t = nc.dram_tensor(f"dbg_{name}", [p] + counts, ap.dtype,
                           kind="ExternalOutput")
        nc.sync.dma_start(t[:], ap)

    with tile.TileContext(nc) as tc:
        with (
            tc.tile_pool(name="consts", bufs=1) as consts,
            tc.tile_pool(name="persist", bufs=1) as persist,
            tc.tile_pool(name="wpool", bufs=1) as wpool,
            tc.tile_pool(name="wstream", bufs=2) as wstream,
            tc.tile_pool(name="tr2", bufs=2) as tr2,
            tc.tile_pool(name="tr1", bufs=1) as tr1,
            tc.tile_pool(name="scanp", bufs=2) as scanp,
            tc.tile_pool(name="reps", bufs=1) as repsp,
            tc.tile_pool(name="dram", bufs=1, space="DRAM") as dramp,
            tc.tile_pool(name="psA", bufs=2, space="PSUM") as psA,
            tc.tile_pool(name="psT", bufs=1, space="PSUM") as psT,
            tc.tile_pool(name="psO", bufs=1, space="PSUM") as psO,
        ):
            # ---------------- constants ----------------
            ident = consts.tile([128, 128], f32)
            make_identity(nc, ident)
            identh = consts.tile([128, 128], f16)
            nc.vector.tensor_copy(identh[:], ident[:])
            g_rep = consts.tile([128, D], f32)
            nc.sync.dma_start(g_rep[:], dram_ap(lng_d, 0, [[0, 128], [1, D]]))
            b_rep = consts.tile([128, D], f32)
            nc.sync.dma_start(b_rep[:], dram_ap(lnb_d, 0, [[0, 128], [1, D]]))
            eps_t = consts.tile([128, 1], f32)
            nc.vector.memset(eps_t[:], 1e-5)
            aux = {}
            for d in ("f", "b"):
                aux[d] = consts.tile([128, NT, 8], f32, tag=f"aux_{d}", name=f"aux_{d}")
                nc.sync.dma_start(
                    aux[d][:],
                    dram_ap(ins[f"aux_{d}"], 0, [[8, 128], [8 * 128, NT], [1, 8]]),
                )

            # ---------------- xT (fp16) via PE transpose ----------------
            # x_d rows are tokens in (l, b) order (host reshaped l-major),
            # so chunk tt == l-slab tt.
            xT = persist.tile([128, KT, TOK], f16, tag="xT")
            for tt in range(ET):
                xtok = tr1.tile([128, D], f32, tag="xtok")
                nc.sync.dma_start(xtok[:], x_d[tt * 128:(tt + 1) * 128, :])
                for ec in range(KT):
                    pst = psT.tile([128, 128], f32, tag="pst")
                    nc.tensor.transpose(pst[:], xtok[:, ec * 128:(ec + 1) * 128], ident[:])
                    nc.scalar.copy(xT[:, ec, tt * 128:(tt + 1) * 128], pst[:])

            dbg("xT", xT[:])
            yout = persist.tile([128, ET, TOK], f16, tag="yout")
            xc = persist.tile([128, NT, TOK], f16, tag="xc")

            ygz_d = {
                d: dramp.tile([NT, 128, TOK], f16, tag=f"ygz_{d}", name=f"ygz_{d}")
                for d in ("f", "b")
            }

            def phase1_mt(d, mt):
                fwd = d == "f"
                if True:
                    win_t = wstream.tile([128, KT, 128], f16, tag="win_t")
                    for kt in range(KT):
                        nc.sync.dma_start(
                            win_t[:, kt, :],
                            dram_ap(ins[f"win_{d}"],
                                    kt * 128 * DI + mt * 128,
                                    [[DI, 128], [1, 128]]),
                        )
                    ps = psA.tile([128, 2, 512], f32, tag="psA")
                    for kt in range(KT):
                        for ng in range(2):
                            nc.tensor.matmul(
                                ps[:, ng, 0:384],
                                win_t[:, kt, :],
                                xT[:, kt, ng * 384:(ng + 1) * 384],
                                start=(kt == 0),
                                stop=(kt == KT - 1),
                            )
                    xi_t = tr2.tile([128, TOK], f16, tag="scr16")
                    for ng in range(2):
                        nc.scalar.copy(xi_t[:, ng * 384:(ng + 1) * 384],
                                       ps[:, ng, 0:384])

                    # conv via PE: diag(cw_k) matmuls, PSUM-accumulated.
                    # fwd: xc[l] = sum_k w[k]*xi[l+k-3]; bwd: sum_j w[3-j]*xi[l+j]
                    wcv = wstream.tile([128, 4, 128], f16, tag="wcv")
                    nc.sync.dma_start(wcv[:], ins[f"cwd_{d}"][mt, :, :, :])
                    ps2 = psA.tile([128, 2, 512], f32, tag="psA")
                    if fwd:
                        taps = {
                            0: [(3, 0, 384, 0), (2, 128, 384, 0), (1, 256, 384, 0)],
                            1: [(3, 0, 384, 384), (2, 0, 384, 256),
                                (1, 0, 384, 128), (0, 0, 384, 0)],
                        }
                    else:
                        taps = {
                            0: [(3, 0, 384, 0), (2, 0, 384, 128),
                                (1, 0, 384, 256), (0, 0, 384, 384)],
                            1: [(3, 0, 384, 384), (2, 0, 256, 512),
                                (1, 0, 128, 640)],
                        }
                    for ng, tl in taps.items():
                        for i, (k, o0, o1, xo) in enumerate(tl):
                            nc.tensor.matmul(
                                ps2[:, ng, o0:o1],
                                wcv[:, k, :],
                                xi_t[:, xo:xo + (o1 - o0)],
                                start=(i == 0), stop=(i == len(tl) - 1),
                            )
                    nc.scalar.activation(
                        out=xc[:, mt, :],
                        in_=ps2[:, :, 0:384],
                        func=AF.Silu,
                        bias=aux[d][:, mt, 4:5],
                    )

            def phase2(d):
                dbg(f"xc_{d}", xc[:])
                # ---------------- phase 2: x_proj -> dt_lo, B, C -----------
                wxp = wpool.tile([128, NT, 80], f16, tag="wxp")
                nc.sync.dma_start(wxp[:], ins[f"wxp_{d}"][:])
                dt_lo = tr1.tile([R, TOK], f16, tag="dt_lo")
                bc_sb = tr1.tile([16, 2, TOK], f16, tag="bc_sb")
                for part, (m0, m1) in enumerate([(0, 48), (48, 64), (64, 80)]):
                    psx = psA.tile([128, 2, 512], f32, tag="psA")
                    for kt in range(NT):
                        for ng in range(2):
                            nc.tensor.matmul(
                                psx[:m1 - m0, ng, 0:384],
                                wxp[:, kt, m0:m1],
                                xc[:, kt, ng * 384:(ng + 1) * 384],
                                start=(kt == 0),
                                stop=(kt == NT - 1),
                            )
                    for ng in range(2):
                        if part == 0:
                            nc.scalar.copy(dt_lo[:, ng * 384:(ng + 1) * 384],
                                           psx[:R, ng, 0:384])
                        else:
                            nc.scalar.copy(
                                bc_sb[:, part - 1, ng * 384:(ng + 1) * 384],
                                psx[:16, ng, 0:384],
                            )

                # stage B/C to DRAM as (half, t, n, b) then broadcast-read
                bstage = dramp.tile([NH, L, NS, BH], f16, tag="bstage")
                cstage = dramp.tile([NH, L, NS, BH], f16, tag="cstage")
                for part, stg in ((0, bstage), (1, cstage)):
                    for h in range(NH):
                        for t in range(L):
                            nc.sync.dma_start(
                                stg[h, t, :, :],
                                bc_sb[:, part, t * 128 + h * BH:
                                      t * 128 + h * BH + BH],
                            )
                brep = repsp.tile([128, NH, L, NS, BH], f16, tag="brep")
                sa = bstage[:, :, :, :]
                nc.sync.dma_start(
                    brep[:],
                    AP(tensor=sa.tensor, offset=sa.offset,
                       ap=[[0, 128], [1, NH * HV]]),
                )
                crep = repsp.tile([128, NH, L, NS, BH], f16, tag="crep")
                sc = cstage[:, :, :, :]
                nc.sync.dma_start(
                    crep[:],
                    AP(tensor=sc.tensor, offset=sc.offset,
                       ap=[[0, 128], [1, NH * HV]]),
                )

                dbg(f"dtlo_{d}", dt_lo[:])
                dbg(f"bc_{d}", bc_sb[:])
                st["dt_lo"] = dt_lo
                st["brep"] = brep
                st["crep"] = crep

            st = {}
            ygated = persist.tile([128, NT, TOK], f16, tag="ygated")

            def phase3(d, after_mt=None):
                """Scan + fused z-gate per mt; dir f DMAs ygated to DRAM."""
                fwd = d == "f"
                dt_lo, brep, crep = st["dt_lo"], st["brep"], st["crep"]
                for mt in range(NT):
                    # dt = softplus(wdt.T @ dt_lo + dt_b), all tokens
                    wdt_t = wstream.tile([R, 128], f16, tag="wdt_t")
                    nc.sync.dma_start(
                        wdt_t[:],
                        dram_ap(ins[f"wdt_{d}"], mt * 128, [[DI, R], [1, 128]]))
                    psd = psA.tile([128, 2, 512], f32, tag="psA")
                    for ng in range(2):
                        nc.tensor.matmul(
                            psd[:, ng, 0:384], wdt_t[:],
                            dt_lo[:, ng * 384:(ng + 1) * 384],
                            start=True, stop=True,
                        )
                    dt16 = tr2.tile([128, TOK], f16, tag="dt16")
                    for ng in range(2):
                        dte = tr2.tile([128, 384], f32, tag="dte")
                        nc.scalar.activation(
                            out=dte[:],
                            in_=psd[:, ng, 0:384], func=AF.Exp,
                            bias=aux[d][:, mt, 5:6],
                        )
                        nc.scalar.activation(
                            out=dt16[:, ng * 384:(ng + 1) * 384],
                            in_=dte[:], func=AF.Ln, bias=1.0,
                        )
                    if mt == 0:
                        dbg(f"dt_{d}", dt16[:])
                    # z-proj for the gate (PE + Silu), independent of scan
                    wz_t = wstream.tile([128, KT, 128], f16, tag="win_t")
                    nc.sync.dma_start(wz_t[:], ins[f"wz_{d}"][mt, :, :, :])
                    psz = psA.tile([128, 2, 512], f32, tag="psA")
                    for kt in range(KT):
                        for ng in range(2):
                            nc.tensor.matmul(
                                psz[:, ng, 0:384], wz_t[:, kt, :],
                                xT[:, kt, ng * 384:(ng + 1) * 384],
                                start=(kt == 0), stop=(kt == KT - 1),
                            )
                    sz_t = tr2.tile([128, TOK], f16, tag="scr16")
                    for ng in range(2):
                        nc.scalar.activation(
                            out=sz_t[:, ng * 384:(ng + 1) * 384],
                            in_=psz[:, ng, 0:384], func=AF.Silu)
                    # wt = dt * xc, (l,b) f16
                    wt_t = tr2.tile([128, TOK], f16, tag="wt_t")
                    nc.vector.tensor_tensor(
                        out=wt_t[:], in0=dt16[:], in1=xc[:, mt, :], op=OP.mult,
                    )

                    powts, wbhs, tmp6s = [], [], []
                    for h in range(NH):
                        # pow[n, (t b)] = exp(-(n+1)*dt) for this b-half
                        powt = scanp.tile([128, NS, L, BH], f16, tag="powt")
                        dt_h = AP(tensor=dt16[:].tensor,
                                  offset=dt16[:].offset + h * BH,
                                  ap=[dt16[:].ap[0], [128, L], [1, BH]])
                        for n in range(8):
                            nc.scalar.activation(
                                out=powt[:, n, :, :], in_=dt_h,
                                func=AF.Exp, scale=-(float(n + 1)),
                            )
                        p8 = powt[:, 7, :, :]
                        nc.vector.tensor_tensor(
                            out=powt[:, 8:16, :, :],
                            in0=AP(tensor=p8.tensor, offset=p8.offset,
                                   ap=[p8.ap[0], [0, 8], [1, L * BH]]),
                            in1=powt[:, 0:8, :, :],
                            op=OP.mult,
                        )
                        if mt == 0:
                            dbg(f"pow_{d}_h{h}", powt[:])
                        # wbh[t, n, b] = wt x brep; becomes h in place.
                        wbh = scanp.tile([128, L, NS, BH], f16, tag="wbh")
                        wt_v = wt_t[:]
                        nc.vector.tensor_tensor(
                            out=wbh[:],
                            in0=AP(tensor=wt_v.tensor,
                                   offset=wt_v.offset + h * BH,
                                   ap=[wt_v.ap[0], [128, L], [0, NS], [1, BH]]),
                            in1=brep[:, h, :, :, :],
                            op=OP.mult,
                        )
                        if mt == 0:
                            dbg(f"wb_{d}_h{h}", wbh[:])
                        powts.append(powt)
                        wbhs.append(wbh)
                        tmp6_h = scanp.tile([128, L, 8, BH], f16, tag="tmp6",
                                            name=f"tmp6_{h}")
                        tmp6s.append(tmp6_h)

                    # recurrence, in place: h[t] = pow[t]*h[prev] + wbh[t]
                    # halves interleaved to fill the dependent-chain bubbles
                    steps = range(1, L) if fwd else range(L - 2, -1, -1)
                    for t in steps:
                        tprev = t - 1 if fwd else t + 1
                        tslots = []
                        for h in range(NH):
                            pv = powts[h][:]
                            pow_t = AP(
                                tensor=pv.tensor, offset=pv.offset + t * BH,
                                ap=[pv.ap[0], [L * BH, NS], [1, BH]])
                            tmpf = tmp6s[h][:].rearrange("p t n b -> p (t n b)")
                            tslot = AP(tensor=tmpf.tensor, offset=tmpf.offset,
                                       ap=[tmpf.ap[0], [1, SL]])
                            tslots.append(tslot)
                            nc.vector.tensor_tensor(
                                out=tslot, in0=pow_t,
                                in1=wbhs[h][:, tprev, :, :].rearrange(
                                    "p n b -> p (n b)"),
                                op=OP.mult,
                            )
                        for h in range(NH):
                            wslab = wbhs[h][:, t, :, :].rearrange(
                                "p n b -> p (n b)")
                            nc.vector.tensor_tensor(
                                out=wslab, in0=tslots[h], in1=wslab, op=OP.add,
                            )
                    if mt == 0:
                        dbg(f"h_{d}_h0", wbhs[0][:])

                    # hc = h * crep in place (DVE); tree L1 on GpSimd
                    wbfs = [wbhs[h][:].rearrange("p t n b -> p (t n b)")
                            for h in range(NH)]
                    tmpfs = [tmp6s[h][:].rearrange("p t n b -> p (t n b)")
                             for h in range(NH)]
                    HSL = 8 * BH

                    def lvl(eng, src, sw, dst, dw, n_out):
                        w = n_out * BH
                        i0 = AP(tensor=src.tensor, offset=src.offset,
                                ap=[src.ap[0], [sw, L], [1, w]])
                        i1 = AP(tensor=src.tensor, offset=src.offset + w,
                                ap=[src.ap[0], [sw, L], [1, w]])
                        o = AP(tensor=dst.tensor, offset=dst.offset,
                               ap=[dst.ap[0], [dw, L], [1, w]])
                        eng.tensor_tensor(out=o, in0=i0, in1=i1, op=OP.add)

                    for h in range(NH):
                        nc.vector.tensor_tensor(
                            out=wbfs[h], in0=wbfs[h],
                            in1=crep[:, h, :, :, :].rearrange(
                                "p t n b -> p (t n b)"),
                            op=OP.mult,
                        )
                        lvl(nc.gpsimd if h == 0 else nc.vector,
                            wbfs[h], SL, tmpfs[h], HSL, 8)
                    yfins = []
                    for h in range(NH):
                        lvl(nc.vector, tmpfs[h], HSL, wbfs[h], SL, 4)
                        lvl(nc.vector, wbfs[h], SL, tmpfs[h], HSL, 2)
                        yfin = scanp.tile([128, L, BH], f16, tag="yfin")
                        yfins.append(yfin)
                        nc.vector.tensor_tensor(
                            out=yfin[:].rearrange("p t b -> p (t b)"),
                            in0=AP(tensor=tmpfs[h].tensor,
                                   offset=tmpfs[h].offset,
                                   ap=[tmpfs[h].ap[0], [HSL, L], [1, BH]]),
                            in1=AP(tensor=tmpfs[h].tensor,
                                   offset=tmpfs[h].offset + BH,
                                   ap=[tmpfs[h].ap[0], [HSL, L], [1, BH]]),
                            op=OP.add,
                        )
                        if mt == 0:
                            dbg(f"yfin_{d}_h{h}", yfin[:])
                    # skip term: ygated slot = D*xc + yfin, then z-gate
                    og = ygated[:, mt, :]
                    for h in range(NH):
                        nc.vector.scalar_tensor_tensor(
                            out=AP(tensor=og.tensor, offset=og.offset + h * BH,
                                   ap=[og.ap[0], [128, L], [1, BH]]),
                            in0=AP(tensor=xc[:, mt, :].tensor,
                                   offset=xc[:, mt, :].offset + h * BH,
                                   ap=[og.ap[0], [128, L], [1, BH]]),
                            scalar=aux[d][:, mt, 6:7],
                            in1=yfins[h][:].rearrange("p t b -> p (t b)"),
                            op0=OP.mult, op1=OP.add,
                        )
                    nc.vector.tensor_tensor(out=og, in0=og, in1=sz_t[:],
                                            op=OP.mult)
                    if d == "f":
                        nc.sync.dma_start(ygz_d[d][mt, :, :], og)
                    if after_mt and mt in after_mt:
                        after_mt[mt]()
                dbg(f"ygated_{d}", ygated[:])

            def phase4_group(d, dir_i, ng, mg, from_dram):
                pso = psO.tile([128, 3, 512], f32, tag="psO")
                for kt in range(NT):
                    wo_t = wstream.tile([128, 3, 128], f16, tag="wo_t")
                    nc.sync.dma_start(
                        wo_t[:],
                        dram_ap(
                            ins[f"wout_{d}"],
                            kt * 128 * D + mg * 384,
                            [[D, 128], [128, 3], [1, 128]],
                        ),
                    )
                    if from_dram:
                        yg_t = wstream.tile([128, 384], f16, tag="yg_t")
                        ygv = ygz_d[d][:, :, :]
                        nc.sync.dma_start(
                            yg_t[:],
                            AP(tensor=ygv.tensor,
                               offset=ygv.offset + kt * 128 * TOK + ng * 384,
                               ap=[[TOK, 128], [1, 384]]),
                        )
                        rhs = yg_t[:]
                    else:
                        rhs = ygated[:, kt, ng * 384:(ng + 1) * 384]
                    for m in range(3):
                        nc.tensor.matmul(
                            pso[:, m, 0:384], wo_t[:, m, :], rhs,
                            start=(kt == 0), stop=(kt == NT - 1),
                        )
                for m in range(3):
                    mt_e = mg * 3 + m
                    o = yout[:, mt_e, ng * 384:(ng + 1) * 384]
                    if dir_i == 0:
                        nc.scalar.copy(o, pso[:, m, 0:384])
                    else:
                        nc.vector.tensor_tensor(
                            out=o, in0=o, in1=pso[:, m, 0:384], op=OP.add)

            # ---- pipelined emission order ----
            for mt in range(NT):
                phase1_mt("f", mt)
            phase2("f")
            ph1b = {mt: (lambda m=mt: phase1_mt("b", m)) for mt in range(NT)}
            phase3("f", after_mt=ph1b)
            phase2("b")
            ph4f = {
                2: lambda: phase4_group("f", 0, 0, 0, True),
                5: lambda: phase4_group("f", 0, 0, 1, True),
                8: lambda: phase4_group("f", 0, 1, 0, True),
                11: lambda: phase4_group("f", 0, 1, 1, True),
            }
            phase3("b", after_mt=ph4f)
            for ng in range(2):
                for mg in range(2):
                    phase4_group("b", 1, ng, mg, False)

            dbg("yout", yout[:])
            # ---------------- phase 5: residual + LN -> out --------------
            for tt in range(ET):
                xtok = tr1.tile([128, D], f32, tag="xtok")
                nc.sync.dma_start(xtok[:], x_d[tt * 128:(tt + 1) * 128, :])
                r_t = tr1.tile([128, D], f32, tag="r_t")
                for ec in range(KT):
                    psh = psT.tile([128, 128], f16, tag="pst")
                    nc.tensor.transpose(
                        psh[:], yout[:, ec, tt * 128:(tt + 1) * 128], identh[:]
                    )
                    nc.vector.tensor_tensor(
                        out=r_t[:, ec * 128:(ec + 1) * 128],
                        in0=psh[:], in1=xtok[:, ec * 128:(ec + 1) * 128],
                        op=OP.add,
                    )
                stats = tr1.tile([128, 3, nc.vector.BN_STATS_DIM], f32, tag="stats")
                for sub in range(3):
                    nc.vector.bn_stats(
                        out=stats[:, sub, :], in_=r_t[:, sub * 256:(sub + 1) * 256]
                    )
                mv = tr1.tile([128, nc.vector.BN_AGGR_DIM], f32, tag="mv")
                nc.vector.bn_aggr(out=mv[:], in_=stats[:])
                rstd = tr1.tile([128, 1], f32, tag="rstd")
                nc.scalar.activation(
                    out=rstd[:], in_=mv[:, 1:2], func=AF.Sqrt, bias=eps_t[:],
                )
                nc.vector.reciprocal(out=rstd[:], in_=rstd[:])
                nc.vector.tensor_scalar(
                    out=r_t[:], in0=r_t[:], scalar1=mv[:, 0:1], scalar2=rstd[:],
                    op0=OP.subtract, op1=OP.mult,
                )
                nc.vector.tensor_tensor(out=r_t[:], in0=r_t[:], in1=g_rep[:], op=OP.mult)
                nc.vector.tensor_tensor(out=r_t[:], in0=r_t[:], in1=b_rep[:], op=OP.add)
                nc.sync.dma_start(out_d[tt * 128:(tt + 1) * 128, :], r_t[:])

    nc.compile()
    return nc


def _prep_inputs(inputs):
    f16 = np.float16
    shared = {}
    for d in ("f", "b"):
        in_proj = np.asarray(inputs[f"{d}_in"], np.float32)      # [3072, 768]
        shared[f"win_{d}"] = np.ascontiguousarray(in_proj[:DI].T).astype(f16)
        wz_T = in_proj[DI:].T                                    # [768, 1536]
        shared[f"wz_{d}"] = np.ascontiguousarray(
            wz_T.reshape(KT, 128, NT, 128).transpose(2, 1, 0, 3)
        ).astype(f16)
        xp_T = np.asarray(inputs[f"{d}_xp"], np.float32).T       # [1536, 80]
        shared[f"wxp_{d}"] = np.ascontiguousarray(
            xp_T.reshape(NT, 128, 80).transpose(1, 0, 2)
        ).astype(f16)
        shared[f"wdt_{d}"] = np.ascontiguousarray(
            np.asarray(inputs[f"{d}_dtw"], np.float32).T
        ).astype(f16)                                            # [48, 1536]
        shared[f"wout_{d}"] = np.ascontiguousarray(
            np.asarray(inputs[f"{d}_out"], np.float32).T
        ).astype(f16)                                            # [1536, 768]
        aux = np.zeros((DI, 8), np.float32)
        aux[:, 0:4] = np.asarray(inputs[f"{d}_cw"], np.float32).T
        aux[:, 4] = np.asarray(inputs[f"{d}_cb"], np.float32)
        aux[:, 5] = np.asarray(inputs[f"{d}_dtb"], np.float32)
        aux[:, 6] = np.asarray(inputs[f"{d}_D"], np.float32)
        shared[f"aux_{d}"] = aux
        cw = np.asarray(inputs[f"{d}_cw"], np.float32)           # [4, DI]
        cwd = np.zeros((NT, 128, 4, 128), np.float32)
        idx = np.arange(128)
        for mt in range(NT):
            for k in range(4):
                cwd[mt, idx, k, idx] = cw[k, mt * 128:(mt + 1) * 128]
        shared[f"cwd_{d}"] = cwd.astype(f16)
    shared["ln_g"] = np.ascontiguousarray(np.asarray(inputs["ln_g"], np.float32))
    shared["ln_b"] = np.ascontiguousarray(np.asarray(inputs["ln_b"], np.float32))
    return shared


def kernel(**inputs):
    from concourse import bass_utils

    if "nc" not in _CACHE:
        _CACHE["nc"] = _build_module()
    nc = _CACHE["nc"]

    shared = _prep_inputs(inputs)
    x = np.asarray(inputs["x"], np.float32)
    n_cores = 8
    bs = x.shape[0] // n_cores

    in_maps = []
    for c in range(n_cores):
        m = dict(shared)
        # l-major token order: row t*B + b
        m["x"] = np.ascontiguousarray(
            x[c * bs:(c + 1) * bs].transpose(1, 0, 2).reshape(TOK, D)
        ).astype(np.float32)
        in_maps.append(m)

    res = bass_utils.run_bass_kernel_spmd(nc, in_maps, core_ids=list(range(n_cores)))
    out = np.concatenate(
        [r["out"].reshape(L, bs, D).transpose(1, 0, 2) for r in res.results],
        axis=0,
    )
    return out.astype(np.float32)


# revision 33
# speedup vs baseline: 1.0630x; 1.0253x over previous
"""Bidirectional Mamba (PartContextMamba) Trainium2 Bass kernel, v2.

Sharding: pure data parallelism over batch (1024 -> 8 cores x 128 batch).
Token order on all free axes is (l, b) -- l OUTER, b inner (host reshapes
x to l-major). This makes every scan-phase access contiguous:

  xT [768d, (l b)] f16 (PE transpose of the x shard)
  per direction (fwd, bwd):
    xi = W_in_xi @ xT (PE), conv via shifted-slab STTs (DVE), silu (ACT)
    x_dbl = W_xp @ xc -> dt_lo[48] f32, B[16], C[16] f16
    B/C staged to DRAM as (half, t, n, b) then partition-broadcast to all
    128 partitions (brep/crep).
    per mt (12 d-tiles of 128):
      dt = softplus(W_dt @ dt_lo + dt_b)  (ACT Exp+Ln, one table)
      wt = dt*xc (DVE)
      per b-half h (64):
        pow[n,t,b] = exp(-(n+1)dt): 8 ACT exps + 1 DVE doubling TT
        wbh[t,n,b] = wt x brep (DVE TT, becomes h in place)
        recurrence h[t] = pow[t]*h[t-1] + wbh[t]: 10 unrolled TTs
        (in-place over wbh; bwd runs the slab loop in reverse)
        hc = h*crep -> pow buffer; log-tree reduce over n (GpSimd TTs,
        ping-pong between wbh/pow buffers) -> yfin f32
        y = yfin + D*xc (STT) -> ygated slot
    z-gate: ygated *= silu(W_z @ xT) (PE + ACT Silu + DVE TT)
    yout += W_out @ ygated (PE, PSUM k-accum)
  out = LayerNorm(x + yout^T) (PE transpose, ACT Rsqrt, token-major)
"""

import numpy as np

_CACHE: dict = {}

B = 128          # batch per core
L = 6
D = 768
DI = 1536
NT = 12          # d-tiles
NS = 16          # ssm states
R = 48           # dt rank
TOK = B * L
ET = 6           # token-tiles (now l-slabs)
KT = 6           # k-tiles of D
NH = 2           # b-halves
BH = B // NH     # 64
HV = NS * BH * L  # 6144 elems per half of brep/crep
SL = NS * BH     # 1024, one t-slab in (t,n,b)


def _build_module(debug=False):
    import concourse.bass as bass
    import concourse.bacc as bacc
    import concourse.mybir as mybir
    import concourse.tile as tile
    from concourse.masks import make_identity

    f32 = mybir.dt.float32
    f16 = mybir.dt.float16
    AP = bass.AP
    AF = mybir.ActivationFunctionType
    OP = mybir.AluOpType

    nc = bacc.Bacc("TRN2", target_bir_lowering=False)

    x_d = nc.dram_tensor("x", [TOK, D], f32, kind="ExternalInput")
    ins = {}
    for d in ("f", "b"):
        ins[f"win_{d}"] = nc.dram_tensor(f"win_{d}", [D, DI], f16, kind="ExternalInput")
        ins[f"wz_{d}"] = nc.dram_tensor(f"wz_{d}", [NT, 128, KT, 128], f16, kind="ExternalInput")
        ins[f"wxp_{d}"] = nc.dram_tensor(f"wxp_{d}", [128, NT, 80], f16, kind="ExternalInput")
        ins[f"wdt_{d}"] = nc.dram_tensor(f"wdt_{d}", [R, DI], f16, kind="ExternalInput")
        ins[f"wout_{d}"] = nc.dram_tensor(f"wout_{d}", [DI, D], f16, kind="ExternalInput")
        ins[f"aux_{d}"] = nc.dram_tensor(f"aux_{d}", [DI, 8], f32, kind="ExternalInput")
        ins[f"cwd_{d}"] = nc.dram_tensor(f"cwd_{d}", [NT, 128, 4, 128], f16, kind="ExternalInput")
    lng_d = nc.dram_tensor("ln_g", [D], f32, kind="ExternalInput")
    lnb_d = nc.dram_tensor("ln_b", [D], f32, kind="ExternalInput")
    out_d = nc.dram_tensor("out", [TOK, D], f32, kind="ExternalOutput")

    def dram_ap(t, offset, ap):
        return AP(tensor=t, offset=offset, ap=ap)

    def dbg(name, ap):
        if not debug:
            return
        p = ap.partition_size()
        counts = [c for _, c in ap.ap[1:]]
        t = nc.dram_tensor(f"dbg_{name}", [p] + counts, ap.dtype,
                           kind="ExternalOutput")
        nc.sync.dma_start(t[:], ap)

    with tile.TileContext(nc) as tc:
        with (
            tc.tile_pool(name="consts", bufs=1) as consts,
            tc.tile_pool(name="persist", bufs=1) as persist,
            tc.tile_pool(name="wpool", bufs=1) as wpool,
            tc.tile_pool(name="wstream", bufs=2) as wstream,
            tc.tile_pool(name="tr2", bufs=2) as tr2,
            tc.tile_pool(name="tr1", bufs=1) as tr1,
            tc.tile_pool(name="scanp", bufs=2) as scanp,
            tc.tile_pool(name="reps", bufs=1) as repsp,
            tc.tile_pool(name="dram", bufs=1, space="DRAM") as dramp,
            tc.tile_pool(name="psA", bufs=2, space="PSUM") as psA,
            tc.tile_pool(name="psT", bufs=1, space="PSUM") as psT,
            tc.tile_pool(name="psO", bufs=1, space="PSUM") as psO,
        ):
            # ---------------- constants ----------------
            ident = consts.tile([128, 128], f32)
            make_identity(nc, ident)
            identh = consts.tile([128, 128], f16)
            nc.vector.tensor_copy(identh[:], ident[:])
            g_rep = consts.tile([128, D], f32)
            nc.sync.dma_start(g_rep[:], dram_ap(lng_d, 0, [[0, 128], [1, D]]))
            b_rep = consts.tile([128, D], f32)
            nc.sync.dma_start(b_rep[:], dram_ap(lnb_d, 0, [[0, 128], [1, D]]))
            eps_t = consts.tile([128, 1], f32)
            nc.vector.memset(eps_t[:], 1e-5)
            aux = {}
            for d in ("f", "b"):
                aux[d] = consts.tile([128, NT, 8], f32, tag=f"aux_{d}", name=f"aux_{d}")
                nc.sync.dma_start(
                    aux[d][:],
                    dram_ap(ins[f"aux_{d}"], 0, [[8, 128], [8 * 128, NT], [1, 8]]),
                )

            # ---------------- xT (fp16) via PE transpose ----------------
            # x_d rows are tokens in (l, b) order (host reshaped l-major),
            # so chunk tt == l-slab tt.
            xT = persist.tile([128, KT, TOK], f16, tag="xT")
            for tt in range(ET):
                xtok = tr1.tile([128, D], f32, tag="xtok")
                nc.sync.dma_start(xtok[:], x_d[tt * 128:(tt + 1) * 128, :])
                pst = psO.tile([128, 3, 512], f32, tag="psO")
                for ec in range(KT):
                    sl = pst[:, ec // 2, (ec % 2) * 128:(ec % 2) * 128 + 128]
                    nc.tensor.transpose(sl, xtok[:, ec * 128:(ec + 1) * 128], ident[:])
                for ec in range(KT):
                    sl = pst[:, ec // 2, (ec % 2) * 128:(ec % 2) * 128 + 128]
                    nc.scalar.copy(xT[:, ec, tt * 128:(tt + 1) * 128], sl)

            dbg("xT", xT[:])
            yout = persist.tile([128, ET, TOK], f16, tag="yout")
            xc = persist.tile([128, NT, TOK], f16, tag="xc")

            ygz_d = {
                d: dramp.tile([NT, 128, TOK], f16, tag=f"ygz_{d}", name=f"ygz_{d}")
                for d in ("f", "b")
            }

            def phase1_mt(d, mt):
                fwd = d == "f"
                if True:
                    win_t = wstream.tile([128, KT, 128], f16, tag="win_t")
                    for kt in range(KT):
                        nc.sync.dma_start(
                            win_t[:, kt, :],
                            dram_ap(ins[f"win_{d}"],
                                    kt * 128 * DI + mt * 128,
                                    [[DI, 128], [1, 128]]),
                        )
                    ps = psA.tile([128, 2, 512], f32, tag="psA")
                    for kt in range(KT):
                        for ng in range(2):
                            nc.tensor.matmul(
                                ps[:, ng, 0:384],
                                win_t[:, kt, :],
                                xT[:, kt, ng * 384:(ng + 1) * 384],
                                start=(kt == 0),
                                stop=(kt == KT - 1),
                            )
                    xi_t = tr2.tile([128, TOK], f16, tag="scr16")
                    for ng in range(2):
                        nc.scalar.copy(xi_t[:, ng * 384:(ng + 1) * 384],
                                       ps[:, ng, 0:384])

                    # conv via PE: diag(cw_k) matmuls, PSUM-accumulated.
                    # fwd: xc[l] = sum_k w[k]*xi[l+k-3]; bwd: sum_j w[3-j]*xi[l+j]
                    wcv = wstream.tile([128, 4, 128], f16, tag="wcv")
                    nc.sync.dma_start(wcv[:], ins[f"cwd_{d}"][mt, :, :, :])
                    ps2 = psA.tile([128, 2, 512], f32, tag="psA")
                    if fwd:
                        taps = {
                            0: [(3, 0, 384, 0), (2, 128, 384, 0), (1, 256, 384, 0)],
                            1: [(3, 0, 384, 384), (2, 0, 384, 256),
                                (1, 0, 384, 128), (0, 0, 384, 0)],
                        }
                    else:
                        taps = {
                            0: [(3, 0, 384, 0), (2, 0, 384, 128),
                                (1, 0, 384, 256), (0, 0, 384, 384)],
                            1: [(3, 0, 384, 384), (2, 0, 256, 512),
                                (1, 0, 128, 640)],
                        }
                    for ng, tl in taps.items():
                        for i, (k, o0, o1, xo) in enumerate(tl):
                            nc.tensor.matmul(
                                ps2[:, ng, o0:o1],
                                wcv[:, k, :],
                                xi_t[:, xo:xo + (o1 - o0)],
                                start=(i == 0), stop=(i == len(tl) - 1),
                            )
                    nc.scalar.activation(
                        out=xc[:, mt, :],
                        in_=ps2[:, :, 0:384],
                        func=AF.Silu,
                        bias=aux[d][:, mt, 4:5],
                    )

            def phase2(d):
                dbg(f"xc_{d}", xc[:])
                # ---------------- phase 2: x_proj -> dt_lo, B, C -----------
                wxp = wpool.tile([128, NT, 80], f16, tag="wxp")
                nc.sync.dma_start(wxp[:], ins[f"wxp_{d}"][:])
                dt_lo = tr1.tile([R, TOK], f16, tag="dt_lo")
                bc_sb = tr1.tile([16, 2, TOK], f16, tag="bc_sb")
                for part, (m0, m1) in enumerate([(0, 48), (48, 64), (64, 80)]):
                    psx = psA.tile([128, 2, 512], f32, tag="psA")
                    for kt in range(NT):
                        for ng in range(2):
                            nc.tensor.matmul(
                                psx[:m1 - m0, ng, 0:384],
                                wxp[:, kt, m0:m1],
                                xc[:, kt, ng * 384:(ng + 1) * 384],
                                start=(kt == 0),
                                stop=(kt == NT - 1),
                            )
                    for ng in range(2):
                        if part == 0:
                            nc.scalar.copy(dt_lo[:, ng * 384:(ng + 1) * 384],
                                           psx[:R, ng, 0:384])
                        else:
                            nc.scalar.copy(
                                bc_sb[:, part - 1, ng * 384:(ng + 1) * 384],
                                psx[:16, ng, 0:384],
                            )

                # stage B/C to DRAM as (half, t, n, b) then broadcast-read
                bstage = dramp.tile([NH, L, NS, BH], f16, tag="bstage")
                cstage = dramp.tile([NH, L, NS, BH], f16, tag="cstage")
                for part, stg in ((0, bstage), (1, cstage)):
                    for h in range(NH):
                        for t in range(L):
                            nc.sync.dma_start(
                                stg[h, t, :, :],
                                bc_sb[:, part, t * 128 + h * BH:
                                      t * 128 + h * BH + BH],
                            )
                brep = repsp.tile([128, NH, L, NS, BH], f16, tag="brep")
                sa = bstage[:, :, :, :]
                crep = repsp.tile([128, NH, L, NS, BH], f16, tag="crep")
                sc = cstage[:, :, :, :]
                for h in range(NH):
                    nc.sync.dma_start(
                        brep[:, h, :, :, :],
                        AP(tensor=sa.tensor, offset=sa.offset + h * HV,
                           ap=[[0, 128], [1, HV]]),
                    )
                    nc.sync.dma_start(
                        crep[:, h, :, :, :],
                        AP(tensor=sc.tensor, offset=sc.offset + h * HV,
                           ap=[[0, 128], [1, HV]]),
                    )

                dbg(f"dtlo_{d}", dt_lo[:])
                dbg(f"bc_{d}", bc_sb[:])
                st["dt_lo"] = dt_lo
                st["brep"] = brep
                st["crep"] = crep

            st = {}
            ygated = persist.tile([128, NT, TOK], f16, tag="ygated")

            def phase3(d, after_mt=None):
                """Scan + fused z-gate per mt; dir f DMAs ygated to DRAM."""
                fwd = d == "f"
                dt_lo, brep, crep = st["dt_lo"], st["brep"], st["crep"]
                for mt in range(NT):
                    # dt = softplus(wdt.T @ dt_lo + dt_b), all tokens
                    wdt_t = wstream.tile([R, 128], f16, tag="wdt_t")
                    nc.sync.dma_start(
                        wdt_t[:],
                        dram_ap(ins[f"wdt_{d}"], mt * 128, [[DI, R], [1, 128]]))
                    psd = psA.tile([128, 2, 512], f32, tag="psA")
                    for ng in range(2):
                        nc.tensor.matmul(
                            psd[:, ng, 0:384], wdt_t[:],
                            dt_lo[:, ng * 384:(ng + 1) * 384],
                            start=True, stop=True,
                        )
                    dt16 = tr2.tile([128, TOK], f16, tag="dt16")
                    for ng in range(2):
                        dte = tr2.tile([128, 384], f32, tag="dte")
                        nc.scalar.activation(
                            out=dte[:],
                            in_=psd[:, ng, 0:384], func=AF.Exp,
                            bias=aux[d][:, mt, 5:6],
                        )
                        nc.scalar.activation(
                            out=dt16[:, ng * 384:(ng + 1) * 384],
                            in_=dte[:], func=AF.Ln, bias=1.0,
                        )
                    if mt == 0:
                        dbg(f"dt_{d}", dt16[:])
                    # z-proj for the gate (PE + Silu), independent of scan
                    wz_t = wstream.tile([128, KT, 128], f16, tag="win_t")
                    nc.sync.dma_start(wz_t[:], ins[f"wz_{d}"][mt, :, :, :])
                    psz = psA.tile([128, 2, 512], f32, tag="psA")
                    for kt in range(KT):
                        for ng in range(2):
                            nc.tensor.matmul(
                                psz[:, ng, 0:384], wz_t[:, kt, :],
                                xT[:, kt, ng * 384:(ng + 1) * 384],
                                start=(kt == 0), stop=(kt == KT - 1),
                            )
                    sz_t = tr2.tile([128, TOK], f16, tag="scr16")
                    for ng in range(2):
                        nc.scalar.activation(
                            out=sz_t[:, ng * 384:(ng + 1) * 384],
                            in_=psz[:, ng, 0:384], func=AF.Silu)
                    # wt = dt * xc, (l,b) f16
                    wt_t = tr2.tile([128, TOK], f16, tag="wt_t")
                    nc.vector.tensor_tensor(
                        out=wt_t[:], in0=dt16[:], in1=xc[:, mt, :], op=OP.mult,
                    )

                    powts, wbhs, tmp6s = [], [], []
                    for h in range(NH):
                        # pow[n, (t b)] = exp(-(n+1)*dt) for this b-half
                        powt = scanp.tile([128, NS, L, BH], f16, tag="powt")
                        dt_h = AP(tensor=dt16[:].tensor,
                                  offset=dt16[:].offset + h * BH,
                                  ap=[dt16[:].ap[0], [128, L], [1, BH]])
                        for n in range(8):
                            nc.scalar.activation(
                                out=powt[:, n, :, :], in_=dt_h,
                                func=AF.Exp, scale=-(float(n + 1)),
                            )
                        p8 = powt[:, 7, :, :]
                        nc.vector.tensor_tensor(
                            out=powt[:, 8:16, :, :],
                            in0=AP(tensor=p8.tensor, offset=p8.offset,
                                   ap=[p8.ap[0], [0, 8], [1, L * BH]]),
                            in1=powt[:, 0:8, :, :],
                            op=OP.mult,
                        )
                        if mt == 0:
                            dbg(f"pow_{d}_h{h}", powt[:])
                        # wbh[t, n, b] = wt x brep; becomes h in place.
                        wbh = scanp.tile([128, L, NS, BH], f16, tag="wbh")
                        wt_v = wt_t[:]
                        nc.vector.tensor_tensor(
                            out=wbh[:],
                            in0=AP(tensor=wt_v.tensor,
                                   offset=wt_v.offset + h * BH,
                                   ap=[wt_v.ap[0], [128, L], [0, NS], [1, BH]]),
                            in1=brep[:, h, :, :, :],
                            op=OP.mult,
                        )
                        if mt == 0:
                            dbg(f"wb_{d}_h{h}", wbh[:])
                        powts.append(powt)
                        wbhs.append(wbh)
                        tmp6_h = scanp.tile([128, L, 8, BH], f16, tag="tmp6",
                                            name=f"tmp6_{h}")
                        tmp6s.append(tmp6_h)

                    # recurrence, in place: h[t] = pow[t]*h[prev] + wbh[t]
                    # halves interleaved to fill the dependent-chain bubbles
                    steps = range(1, L) if fwd else range(L - 2, -1, -1)
                    for t in steps:
                        tprev = t - 1 if fwd else t + 1
                        tslots = []
                        for h in range(NH):
                            pv = powts[h][:]
                            pow_t = AP(
                                tensor=pv.tensor, offset=pv.offset + t * BH,
                                ap=[pv.ap[0], [L * BH, NS], [1, BH]])
                            tmpf = tmp6s[h][:].rearrange("p t n b -> p (t n b)")
                            tslot = AP(tensor=tmpf.tensor, offset=tmpf.offset,
                                       ap=[tmpf.ap[0], [1, SL]])
                            tslots.append(tslot)
                            nc.vector.tensor_tensor(
                                out=tslot, in0=pow_t,
                                in1=wbhs[h][:, tprev, :, :].rearrange(
                                    "p n b -> p (n b)"),
                                op=OP.mult,
                            )
                        for h in range(NH):
                            wslab = wbhs[h][:, t, :, :].rearrange(
                                "p n b -> p (n b)")
                            nc.vector.tensor_tensor(
                                out=wslab, in0=tslots[h], in1=wslab, op=OP.add,
                            )
                    if mt == 0:
                        dbg(f"h_{d}_h0", wbhs[0][:])

                    # hc = h * crep in place (DVE); tree L1 on GpSimd
                    wbfs = [wbhs[h][:].rearrange("p t n b -> p (t n b)")
                            for h in range(NH)]
                    tmpfs = [tmp6s[h][:].rearrange("p t n b -> p (t n b)")
                             for h in range(NH)]
                    HSL = 8 * BH

                    def lvl(eng, src, sw, dst, dw, n_out):
                        w = n_out * BH
                        i0 = AP(tensor=src.tensor, offset=src.offset,
                                ap=[src.ap[0], [sw, L], [1, w]])
                        i1 = AP(tensor=src.tensor, offset=src.offset + w,
                                ap=[src.ap[0], [sw, L], [1, w]])
                        o = AP(tensor=dst.tensor, offset=dst.offset,
                               ap=[dst.ap[0], [dw, L], [1, w]])
                        eng.tensor_tensor(out=o, in0=i0, in1=i1, op=OP.add)

                    for h in range(NH):
                        nc.vector.tensor_tensor(
                            out=wbfs[h], in0=wbfs[h],
                            in1=crep[:, h, :, :, :].rearrange(
                                "p t n b -> p (t n b)"),
                            op=OP.mult,
                        )
                        lvl(nc.gpsimd if h == 0 else nc.vector,
                            wbfs[h], SL, tmpfs[h], HSL, 8)
                    yfins = []
                    for h in range(NH):
                        lvl(nc.vector, tmpfs[h], HSL, wbfs[h], SL, 4)
                        lvl(nc.vector, wbfs[h], SL, tmpfs[h], HSL, 2)
                        yfin = scanp.tile([128, L, BH], f16, tag="yfin")
                        yfins.append(yfin)
                        nc.vector.tensor_tensor(
                            out=yfin[:].rearrange("p t b -> p (t b)"),
                            in0=AP(tensor=tmpfs[h].tensor,
                                   offset=tmpfs[h].offset,
                                   ap=[tmpfs[h].ap[0], [HSL, L], [1, BH]]),
                            in1=AP(tensor=tmpfs[h].tensor,
                                   offset=tmpfs[h].offset + BH,
                                   ap=[tmpfs[h].ap[0], [HSL, L], [1, BH]]),
                            op=OP.add,
                        )
                        if mt == 0:
                            dbg(f"yfin_{d}_h{h}", yfin[:])
                    # skip term: ygated slot = D*xc + yfin, then z-gate
                    og = ygated[:, mt, :]
                    for h in range(NH):
                        nc.vector.scalar_tensor_tensor(
                            out=AP(tensor=og.tensor, offset=og.offset + h * BH,
                                   ap=[og.ap[0], [128, L], [1, BH]]),
                            in0=AP(tensor=xc[:, mt, :].tensor,
                                   offset=xc[:, mt, :].offset + h * BH,
                                   ap=[og.ap[0], [128, L], [1, BH]]),
                            scalar=aux[d][:, mt, 6:7],
                            in1=yfins[h][:].rearrange("p t b -> p (t b)"),
                            op0=OP.mult, op1=OP.add,
                        )
                    nc.vector.tensor_tensor(out=og, in0=og, in1=sz_t[:],
                                            op=OP.mult)
                    if d == "f":
                        nc.sync.dma_start(ygz_d[d][mt, :, :], og)
                    if after_mt and mt in after_mt:
                        after_mt[mt]()
                dbg(f"ygated_{d}", ygated[:])

            def phase4_group(d, dir_i, ng, mg, from_dram):
                pso = psO.tile([128, 3, 512], f32, tag="psO")
                for kt in range(NT):
                    wo_t = wstream.tile([128, 3, 128], f16, tag="wo_t")
                    nc.sync.dma_start(
                        wo_t[:],
                        dram_ap(
                            ins[f"wout_{d}"],
                            kt * 128 * D + mg * 384,
                            [[D, 128], [128, 3], [1, 128]],
                        ),
                    )
                    if from_dram:
                        yg_t = wstream.tile([128, 384], f16, tag="yg_t")
                        ygv = ygz_d[d][:, :, :]
                        nc.sync.dma_start(
                            yg_t[:],
                            AP(tensor=ygv.tensor,
                               offset=ygv.offset + kt * 128 * TOK + ng * 384,
                               ap=[[TOK, 128], [1, 384]]),
                        )
                        rhs = yg_t[:]
                    else:
                        rhs = ygated[:, kt, ng * 384:(ng + 1) * 384]
                    for m in range(3):
                        nc.tensor.matmul(
                            pso[:, m, 0:384], wo_t[:, m, :], rhs,
                            start=(kt == 0), stop=(kt == NT - 1),
                        )
                for m in range(3):
                    mt_e = mg * 3 + m
                    o = yout[:, mt_e, ng * 384:(ng + 1) * 384]
                    if dir_i == 0:
                        nc.scalar.copy(o, pso[:, m, 0:384])
                    else:
                        nc.vector.tensor_tensor(
                            out=o, in0=o, in1=pso[:, m, 0:384], op=OP.add)

            # ---- pipelined emission order ----
            for mt in range(NT):
                phase1_mt("f", mt)
            phase2("f")
            ph1b = {mt: (lambda m=mt: phase1_mt("b", m)) for mt in range(NT)}
            phase3("f", after_mt=ph1b)
            phase2("b")
            ph4f = {
                2: lambda: phase4_group("f", 0, 0, 0, True),
                5: lambda: phase4_group("f", 0, 0, 1, True),
                8: lambda: phase4_group("f", 0, 1, 0, True),
                11: lambda: phase4_group("f", 0, 1, 1, True),
            }
            phase3("b", after_mt=ph4f)
            for ng in range(2):
                for mg in range(2):
                    phase4_group("b", 1, ng, mg, False)

            dbg("yout", yout[:])
            # ---------------- phase 5: residual + LN -> out --------------
            for tt in range(ET):
                xtok = tr1.tile([128, D], f32, tag="xtok")
                nc.sync.dma_start(xtok[:], x_d[tt * 128:(tt + 1) * 128, :])
                r_t = tr1.tile([128, D], f32, tag="r_t")
                for eg, cnt in ((0, 4), (4, 2)):
                    psh = psT.tile([128, 4, 128], f16, tag="pst")
                    for e in range(cnt):
                        nc.tensor.transpose(
                            psh[:, e, :],
                            yout[:, eg + e, tt * 128:(tt + 1) * 128], identh[:]
                        )
                    for e in range(cnt):
                        ec = eg + e
                        nc.vector.tensor_tensor(
                            out=r_t[:, ec * 128:(ec + 1) * 128],
                            in0=psh[:, e, :], in1=xtok[:, ec * 128:(ec + 1) * 128],
                            op=OP.add,
                        )
                stats = tr1.tile([128, 3, nc.vector.BN_STATS_DIM], f32, tag="stats")
                for sub in range(3):
                    nc.vector.bn_stats(
                        out=stats[:, sub, :], in_=r_t[:, sub * 256:(sub + 1) * 256]
                    )
                mv = tr1.tile([128, nc.vector.BN_AGGR_DIM], f32, tag="mv")
                nc.vector.bn_aggr(out=mv[:], in_=stats[:])
                rstd = tr1.tile([128, 1], f32, tag="rstd")
                nc.scalar.activation(
                    out=rstd[:], in_=mv[:, 1:2], func=AF.Sqrt, bias=eps_t[:],
                )
                nc.vector.reciprocal(out=rstd[:], in_=rstd[:])
                nc.vector.tensor_scalar(
                    out=r_t[:], in0=r_t[:], scalar1=mv[:, 0:1], scalar2=rstd[:],
                    op0=OP.subtract, op1=OP.mult,
                )
                nc.vector.tensor_tensor(out=r_t[:], in0=r_t[:], in1=g_rep[:], op=OP.mult)
                nc.vector.tensor_tensor(out=r_t[:], in0=r_t[:], in1=b_rep[:], op=OP.add)
                nc.sync.dma_start(out_d[tt * 128:(tt + 1) * 128, :], r_t[:])

    nc.compile()
    return nc


def _prep_inputs(inputs):
    f16 = np.float16
    shared = {}
    for d in ("f", "b"):
        in_proj = np.asarray(inputs[f"{d}_in"], np.float32)      # [3072, 768]
        shared[f"win_{d}"] = np.ascontiguousarray(in_proj[:DI].T).astype(f16)
        wz_T = in_proj[DI:].T                                    # [768, 1536]
        shared[f"wz_{d}"] = np.ascontiguousarray(
            wz_T.reshape(KT, 128, NT, 128).transpose(2, 1, 0, 3)
        ).astype(f16)
        xp_T = np.asarray(inputs[f"{d}_xp"], np.float32).T       # [1536, 80]
        shared[f"wxp_{d}"] = np.ascontiguousarray(
            xp_T.reshape(NT, 128, 80).transpose(1, 0, 2)
        ).astype(f16)
        shared[f"wdt_{d}"] = np.ascontiguousarray(
            np.asarray(inputs[f"{d}_dtw"], np.float32).T
        ).astype(f16)                                            # [48, 1536]
        shared[f"wout_{d}"] = np.ascontiguousarray(
            np.asarray(inputs[f"{d}_out"], np.float32).T
        ).astype(f16)                                            # [1536, 768]
        aux = np.zeros((DI, 8), np.float32)
        aux[:, 0:4] = np.asarray(inputs[f"{d}_cw"], np.float32).T
        aux[:, 4] = np.asarray(inputs[f"{d}_cb"], np.float32)
        aux[:, 5] = np.asarray(inputs[f"{d}_dtb"], np.float32)
        aux[:, 6] = np.asarray(inputs[f"{d}_D"], np.float32)
        shared[f"aux_{d}"] = aux
        cw = np.asarray(inputs[f"{d}_cw"], np.float32)           # [4, DI]
        cwd = np.zeros((NT, 128, 4, 128), np.float32)
        idx = np.arange(128)
        for mt in range(NT):
            for k in range(4):
                cwd[mt, idx, k, idx] = cw[k, mt * 128:(mt + 1) * 128]
        shared[f"cwd_{d}"] = cwd.astype(f16)
    shared["ln_g"] = np.ascontiguousarray(np.asarray(inputs["ln_g"], np.float32))
    shared["ln_b"] = np.ascontiguousarray(np.asarray(inputs["ln_b"], np.float32))
    return shared


def kernel(**inputs):
    from concourse import bass_utils

    if "nc" not in _CACHE:
        _CACHE["nc"] = _build_module()
    nc = _CACHE["nc"]

    shared = _prep_inputs(inputs)
    x = np.asarray(inputs["x"], np.float32)
    n_cores = 8
    bs = x.shape[0] // n_cores

    in_maps = []
    for c in range(n_cores):
        m = dict(shared)
        # l-major token order: row t*B + b
        m["x"] = np.ascontiguousarray(
            x[c * bs:(c + 1) * bs].transpose(1, 0, 2).reshape(TOK, D)
        ).astype(np.float32)
        in_maps.append(m)

    res = bass_utils.run_bass_kernel_spmd(nc, in_maps, core_ids=list(range(n_cores)))
    out = np.concatenate(
        [r["out"].reshape(L, bs, D).transpose(1, 0, 2) for r in res.results],
        axis=0,
    )
    return out.astype(np.float32)


# revision 34
# speedup vs baseline: 1.1525x; 1.0843x over previous
"""Bidirectional Mamba (PartContextMamba) Trainium2 Bass kernel, v2.

Sharding: pure data parallelism over batch (1024 -> 8 cores x 128 batch).
Token order on all free axes is (l, b) -- l OUTER, b inner (host reshapes
x to l-major). This makes every scan-phase access contiguous:

  xT [768d, (l b)] f16 (PE transpose of the x shard)
  per direction (fwd, bwd):
    xi = W_in_xi @ xT (PE), conv via shifted-slab STTs (DVE), silu (ACT)
    x_dbl = W_xp @ xc -> dt_lo[48] f32, B[16], C[16] f16
    B/C staged to DRAM as (half, t, n, b) then partition-broadcast to all
    128 partitions (brep/crep).
    per mt (12 d-tiles of 128):
      dt = softplus(W_dt @ dt_lo + dt_b)  (ACT Exp+Ln, one table)
      wt = dt*xc (DVE)
      per b-half h (64):
        pow[n,t,b] = exp(-(n+1)dt): 8 ACT exps + 1 DVE doubling TT
        wbh[t,n,b] = wt x brep (DVE TT, becomes h in place)
        recurrence h[t] = pow[t]*h[t-1] + wbh[t]: 10 unrolled TTs
        (in-place over wbh; bwd runs the slab loop in reverse)
        hc = h*crep -> pow buffer; log-tree reduce over n (GpSimd TTs,
        ping-pong between wbh/pow buffers) -> yfin f32
        y = yfin + D*xc (STT) -> ygated slot
    z-gate: ygated *= silu(W_z @ xT) (PE + ACT Silu + DVE TT)
    yout += W_out @ ygated (PE, PSUM k-accum)
  out = LayerNorm(x + yout^T) (PE transpose, ACT Rsqrt, token-major)
"""

import numpy as np

_CACHE: dict = {}

B = 128          # batch per core
L = 6
D = 768
DI = 1536
NT = 12          # d-tiles
NS = 16          # ssm states
R = 48           # dt rank
TOK = B * L
ET = 6           # token-tiles (now l-slabs)
KT = 6           # k-tiles of D
NH = 2           # b-halves
BH = B // NH     # 64
HV = NS * BH * L  # 6144 elems per half of brep/crep
SL = NS * BH     # 1024, one t-slab in (t,n,b)


def _build_module(debug=False):
    import concourse.bass as bass
    import concourse.bacc as bacc
    import concourse.mybir as mybir
    import concourse.tile as tile
    from concourse.masks import make_identity

    f32 = mybir.dt.float32
    f16 = mybir.dt.float16
    AP = bass.AP
    AF = mybir.ActivationFunctionType
    OP = mybir.AluOpType

    nc = bacc.Bacc("TRN2", target_bir_lowering=False)

    x_d = nc.dram_tensor("x", [TOK, D], f32, kind="ExternalInput")
    ins = {}
    for d in ("f", "b"):
        ins[f"win_{d}"] = nc.dram_tensor(f"win_{d}", [D, DI], f16, kind="ExternalInput")
        ins[f"wz_{d}"] = nc.dram_tensor(f"wz_{d}", [NT, 128, KT, 128], f16, kind="ExternalInput")
        ins[f"wxp_{d}"] = nc.dram_tensor(f"wxp_{d}", [128, NT, 80], f16, kind="ExternalInput")
        ins[f"wdt_{d}"] = nc.dram_tensor(f"wdt_{d}", [R, DI], f16, kind="ExternalInput")
        ins[f"wout_{d}"] = nc.dram_tensor(f"wout_{d}", [DI, D], f16, kind="ExternalInput")
        ins[f"aux_{d}"] = nc.dram_tensor(f"aux_{d}", [DI, 8], f32, kind="ExternalInput")
        ins[f"cwd_{d}"] = nc.dram_tensor(f"cwd_{d}", [NT, 128, 4, 128], f16, kind="ExternalInput")
    lng_d = nc.dram_tensor("ln_g", [D], f32, kind="ExternalInput")
    lnb_d = nc.dram_tensor("ln_b", [D], f32, kind="ExternalInput")
    out_d = nc.dram_tensor("out", [TOK, D], f32, kind="ExternalOutput")

    def dram_ap(t, offset, ap):
        return AP(tensor=t, offset=offset, ap=ap)

    def dbg(name, ap):
        if not debug:
            return
        p = ap.partition_size()
        counts = [c for _, c in ap.ap[1:]]
        t = nc.dram_tensor(f"dbg_{name}", [p] + counts, ap.dtype,
                           kind="ExternalOutput")
        nc.sync.dma_start(t[:], ap)

    with tile.TileContext(nc) as tc:
        with (
            tc.tile_pool(name="consts", bufs=1) as consts,
            tc.tile_pool(name="persist", bufs=1) as persist,
            tc.tile_pool(name="wpool", bufs=1) as wpool,
            tc.tile_pool(name="wstream", bufs=2) as wstream,
            tc.tile_pool(name="tr2", bufs=2) as tr2,
            tc.tile_pool(name="tr1", bufs=1) as tr1,
            tc.tile_pool(name="scanp", bufs=2) as scanp,
            tc.tile_pool(name="reps", bufs=1) as repsp,
            tc.tile_pool(name="dram", bufs=1, space="DRAM") as dramp,
            tc.tile_pool(name="psA", bufs=2, space="PSUM") as psA,
            tc.tile_pool(name="psT", bufs=1, space="PSUM") as psT,
            tc.tile_pool(name="psO", bufs=1, space="PSUM") as psO,
        ):
            # ---------------- constants ----------------
            ident = consts.tile([128, 128], f32)
            make_identity(nc, ident)
            identh = consts.tile([128, 128], f16)
            nc.vector.tensor_copy(identh[:], ident[:])
            g_rep = consts.tile([128, D], f32)
            nc.sync.dma_start(g_rep[:], dram_ap(lng_d, 0, [[0, 128], [1, D]]))
            b_rep = consts.tile([128, D], f32)
            nc.sync.dma_start(b_rep[:], dram_ap(lnb_d, 0, [[0, 128], [1, D]]))
            eps_t = consts.tile([128, 1], f32)
            nc.vector.memset(eps_t[:], 1e-5)
            aux = {}
            for d in ("f", "b"):
                aux[d] = consts.tile([128, NT, 8], f32, tag=f"aux_{d}", name=f"aux_{d}")
                nc.sync.dma_start(
                    aux[d][:],
                    dram_ap(ins[f"aux_{d}"], 0, [[8, 128], [8 * 128, NT], [1, 8]]),
                )

            # ---------------- xT (fp16) via PE transpose ----------------
            # x_d rows are tokens in (l, b) order (host reshaped l-major),
            # so chunk tt == l-slab tt.
            xT = persist.tile([128, KT, TOK], f16, tag="xT")
            for tt in range(ET):
                xtok = tr1.tile([128, D], f32, tag="xtok")
                nc.sync.dma_start(xtok[:], x_d[tt * 128:(tt + 1) * 128, :])
                pst = psO.tile([128, 3, 512], f32, tag="psO")
                for ec in range(KT):
                    sl = pst[:, ec // 2, (ec % 2) * 128:(ec % 2) * 128 + 128]
                    nc.tensor.transpose(sl, xtok[:, ec * 128:(ec + 1) * 128], ident[:])
                for ec in range(KT):
                    sl = pst[:, ec // 2, (ec % 2) * 128:(ec % 2) * 128 + 128]
                    nc.scalar.copy(xT[:, ec, tt * 128:(tt + 1) * 128], sl)

            dbg("xT", xT[:])
            yout = persist.tile([128, ET, TOK], f16, tag="yout")
            xc = persist.tile([128, NT, TOK], f16, tag="xc")

            ygz_d = {
                d: dramp.tile([NT, 128, TOK], f16, tag=f"ygz_{d}", name=f"ygz_{d}")
                for d in ("f", "b")
            }

            def phase1_mt(d, mt):
                fwd = d == "f"
                if True:
                    win_t = wstream.tile([128, KT, 128], f16, tag="win_t")
                    for kt in range(KT):
                        nc.sync.dma_start(
                            win_t[:, kt, :],
                            dram_ap(ins[f"win_{d}"],
                                    kt * 128 * DI + mt * 128,
                                    [[DI, 128], [1, 128]]),
                        )
                    ps = psA.tile([128, 2, 512], f32, tag="psA")
                    for kt in range(KT):
                        for ng in range(2):
                            nc.tensor.matmul(
                                ps[:, ng, 0:384],
                                win_t[:, kt, :],
                                xT[:, kt, ng * 384:(ng + 1) * 384],
                                start=(kt == 0),
                                stop=(kt == KT - 1),
                            )
                    xi_t = tr2.tile([128, TOK], f16, tag="scr16")
                    for ng in range(2):
                        nc.scalar.copy(xi_t[:, ng * 384:(ng + 1) * 384],
                                       ps[:, ng, 0:384])

                    # conv via PE: diag(cw_k) matmuls, PSUM-accumulated.
                    # fwd: xc[l] = sum_k w[k]*xi[l+k-3]; bwd: sum_j w[3-j]*xi[l+j]
                    wcv = wstream.tile([128, 4, 128], f16, tag="wcv")
                    nc.sync.dma_start(wcv[:], ins[f"cwd_{d}"][mt, :, :, :])
                    ps2 = psA.tile([128, 2, 512], f32, tag="psA")
                    if fwd:
                        taps = {
                            0: [(3, 0, 384, 0), (2, 128, 384, 0), (1, 256, 384, 0)],
                            1: [(3, 0, 384, 384), (2, 0, 384, 256),
                                (1, 0, 384, 128), (0, 0, 384, 0)],
                        }
                    else:
                        taps = {
                            0: [(3, 0, 384, 0), (2, 0, 384, 128),
                                (1, 0, 384, 256), (0, 0, 384, 384)],
                            1: [(3, 0, 384, 384), (2, 0, 256, 512),
                                (1, 0, 128, 640)],
                        }
                    for ng, tl in taps.items():
                        for i, (k, o0, o1, xo) in enumerate(tl):
                            nc.tensor.matmul(
                                ps2[:, ng, o0:o1],
                                wcv[:, k, :],
                                xi_t[:, xo:xo + (o1 - o0)],
                                start=(i == 0), stop=(i == len(tl) - 1),
                            )
                    nc.scalar.activation(
                        out=xc[:, mt, :],
                        in_=ps2[:, :, 0:384],
                        func=AF.Silu,
                        bias=aux[d][:, mt, 4:5],
                    )

            def phase2(d):
                dbg(f"xc_{d}", xc[:])
                # ---------------- phase 2: x_proj -> dt_lo, B, C -----------
                wxp = wpool.tile([128, NT, 80], f16, tag="wxp")
                nc.sync.dma_start(wxp[:], ins[f"wxp_{d}"][:])
                dt_lo = tr1.tile([R, TOK], f16, tag="dt_lo")
                bc_sb = tr1.tile([16, 2, TOK], f16, tag="bc_sb")
                for part, (m0, m1) in enumerate([(0, 48), (48, 64), (64, 80)]):
                    psx = psA.tile([128, 2, 512], f32, tag="psA")
                    for kt in range(NT):
                        for ng in range(2):
                            nc.tensor.matmul(
                                psx[:m1 - m0, ng, 0:384],
                                wxp[:, kt, m0:m1],
                                xc[:, kt, ng * 384:(ng + 1) * 384],
                                start=(kt == 0),
                                stop=(kt == NT - 1),
                            )
                    for ng in range(2):
                        if part == 0:
                            nc.scalar.copy(dt_lo[:, ng * 384:(ng + 1) * 384],
                                           psx[:R, ng, 0:384])
                        else:
                            nc.scalar.copy(
                                bc_sb[:, part - 1, ng * 384:(ng + 1) * 384],
                                psx[:16, ng, 0:384],
                            )

                # stage B/C to DRAM as (half, t, n, b) then broadcast-read
                bstage = dramp.tile([NH, L, NS, BH], f16, tag="bstage")
                cstage = dramp.tile([NH, L, NS, BH], f16, tag="cstage")
                for part, stg in ((0, bstage), (1, cstage)):
                    for h in range(NH):
                        for t in range(L):
                            nc.sync.dma_start(
                                stg[h, t, :, :],
                                bc_sb[:, part, t * 128 + h * BH:
                                      t * 128 + h * BH + BH],
                            )
                brep = repsp.tile([128, NH, L, NS, BH], f16, tag="brep")
                sa = bstage[:, :, :, :]
                crep = repsp.tile([128, NH, L, NS, BH], f16, tag="crep")
                sc = cstage[:, :, :, :]
                for h in range(NH):
                    nc.sync.dma_start(
                        brep[:, h, :, :, :],
                        AP(tensor=sa.tensor, offset=sa.offset + h * HV,
                           ap=[[0, 128], [1, HV]]),
                    )
                    nc.sync.dma_start(
                        crep[:, h, :, :, :],
                        AP(tensor=sc.tensor, offset=sc.offset + h * HV,
                           ap=[[0, 128], [1, HV]]),
                    )

                dbg(f"dtlo_{d}", dt_lo[:])
                dbg(f"bc_{d}", bc_sb[:])
                st["dt_lo"] = dt_lo
                st["brep"] = brep
                st["crep"] = crep

            st = {}
            ygated = persist.tile([128, NT, TOK], f16, tag="ygated")

            def phase3(d, after_mt=None):
                """Scan + fused z-gate per mt; dir f DMAs ygated to DRAM."""
                fwd = d == "f"
                dt_lo, brep, crep = st["dt_lo"], st["brep"], st["crep"]
                for mt in range(NT):
                    # dt = softplus(wdt.T @ dt_lo + dt_b), all tokens
                    wdt_t = wstream.tile([R, 128], f16, tag="wdt_t")
                    nc.sync.dma_start(
                        wdt_t[:],
                        dram_ap(ins[f"wdt_{d}"], mt * 128, [[DI, R], [1, 128]]))
                    psd = psA.tile([128, 2, 512], f32, tag="psA")
                    for ng in range(2):
                        nc.tensor.matmul(
                            psd[:, ng, 0:384], wdt_t[:],
                            dt_lo[:, ng * 384:(ng + 1) * 384],
                            start=True, stop=True,
                        )
                    dt16 = tr2.tile([128, TOK], f16, tag="dt16")
                    for ng in range(2):
                        dte = tr2.tile([128, 384], f32, tag="dte")
                        nc.scalar.activation(
                            out=dte[:],
                            in_=psd[:, ng, 0:384], func=AF.Exp,
                            bias=aux[d][:, mt, 5:6],
                        )
                        nc.scalar.activation(
                            out=dt16[:, ng * 384:(ng + 1) * 384],
                            in_=dte[:], func=AF.Ln, bias=1.0,
                        )
                    if mt == 0:
                        dbg(f"dt_{d}", dt16[:])
                    # z-proj for the gate (PE + Silu), independent of scan
                    wz_t = wstream.tile([128, KT, 128], f16, tag="win_t")
                    nc.sync.dma_start(wz_t[:], ins[f"wz_{d}"][mt, :, :, :])
                    psz = psA.tile([128, 2, 512], f32, tag="psA")
                    for kt in range(KT):
                        for ng in range(2):
                            nc.tensor.matmul(
                                psz[:, ng, 0:384], wz_t[:, kt, :],
                                xT[:, kt, ng * 384:(ng + 1) * 384],
                                start=(kt == 0), stop=(kt == KT - 1),
                            )
                    sz_t = tr2.tile([128, TOK], f16, tag="scr16")
                    for ng in range(2):
                        nc.scalar.activation(
                            out=sz_t[:, ng * 384:(ng + 1) * 384],
                            in_=psz[:, ng, 0:384], func=AF.Silu)
                    # wt = dt * xc, (l,b) f16
                    wt_t = tr2.tile([128, TOK], f16, tag="wt_t")
                    nc.vector.tensor_tensor(
                        out=wt_t[:], in0=dt16[:], in1=xc[:, mt, :], op=OP.mult,
                    )

                    powts, wbhs, tmp6s = [], [], []
                    for h in range(NH):
                        # pow[n, (t b)] = exp(-(n+1)*dt) for this b-half
                        powt = scanp.tile([128, NS, L, BH], f16, tag="powt")
                        dt_h = AP(tensor=dt16[:].tensor,
                                  offset=dt16[:].offset + h * BH,
                                  ap=[dt16[:].ap[0], [128, L], [1, BH]])
                        for n in range(8):
                            nc.scalar.activation(
                                out=powt[:, n, :, :], in_=dt_h,
                                func=AF.Exp, scale=-(float(n + 1)),
                            )
                        p8 = powt[:, 7, :, :]
                        nc.vector.tensor_tensor(
                            out=powt[:, 8:16, :, :],
                            in0=AP(tensor=p8.tensor, offset=p8.offset,
                                   ap=[p8.ap[0], [0, 8], [1, L * BH]]),
                            in1=powt[:, 0:8, :, :],
                            op=OP.mult,
                        )
                        if mt == 0:
                            dbg(f"pow_{d}_h{h}", powt[:])
                        # wbh[t, n, b] = wt x brep; becomes h in place.
                        wbh = scanp.tile([128, L, NS, BH], f16, tag="wbh")
                        wt_v = wt_t[:]
                        nc.vector.tensor_tensor(
                            out=wbh[:],
                            in0=AP(tensor=wt_v.tensor,
                                   offset=wt_v.offset + h * BH,
                                   ap=[wt_v.ap[0], [128, L], [0, NS], [1, BH]]),
                            in1=brep[:, h, :, :, :],
                            op=OP.mult,
                        )
                        if mt == 0:
                            dbg(f"wb_{d}_h{h}", wbh[:])
                        powts.append(powt)
                        wbhs.append(wbh)
                        tmp6_h = scanp.tile([128, L, 8, BH], f16, tag="tmp6",
                                            name=f"tmp6_{h}")
                        tmp6s.append(tmp6_h)

                    # recurrence, in place: h[t] = pow[t]*h[prev] + wbh[t]
                    # halves interleaved to fill the dependent-chain bubbles
                    steps = range(1, L) if fwd else range(L - 2, -1, -1)
                    for t in steps:
                        tprev = t - 1 if fwd else t + 1
                        tslots = []
                        for h in range(NH):
                            pv = powts[h][:]
                            pow_t = AP(
                                tensor=pv.tensor, offset=pv.offset + t * BH,
                                ap=[pv.ap[0], [L * BH, NS], [1, BH]])
                            tmpf = tmp6s[h][:].rearrange("p t n b -> p (t n b)")
                            tslot = AP(tensor=tmpf.tensor, offset=tmpf.offset,
                                       ap=[tmpf.ap[0], [1, SL]])
                            tslots.append(tslot)
                            nc.vector.tensor_tensor(
                                out=tslot, in0=pow_t,
                                in1=wbhs[h][:, tprev, :, :].rearrange(
                                    "p n b -> p (n b)"),
                                op=OP.mult,
                            )
                        for h in range(NH):
                            wslab = wbhs[h][:, t, :, :].rearrange(
                                "p n b -> p (n b)")
                            nc.vector.tensor_tensor(
                                out=wslab, in0=tslots[h], in1=wslab, op=OP.add,
                            )
                    if mt == 0:
                        dbg(f"h_{d}_h0", wbhs[0][:])

                    # hc = h * crep in place (DVE); tree L1 on GpSimd
                    wbfs = [wbhs[h][:].rearrange("p t n b -> p (t n b)")
                            for h in range(NH)]
                    tmpfs = [tmp6s[h][:].rearrange("p t n b -> p (t n b)")
                             for h in range(NH)]
                    HSL = 8 * BH

                    def lvl(eng, src, sw, dst, dw, n_out):
                        w = n_out * BH
                        i0 = AP(tensor=src.tensor, offset=src.offset,
                                ap=[src.ap[0], [sw, L], [1, w]])
                        i1 = AP(tensor=src.tensor, offset=src.offset + w,
                                ap=[src.ap[0], [sw, L], [1, w]])
                        o = AP(tensor=dst.tensor, offset=dst.offset,
                               ap=[dst.ap[0], [dw, L], [1, w]])
                        eng.tensor_tensor(out=o, in0=i0, in1=i1, op=OP.add)

                    for h in range(NH):
                        nc.vector.tensor_tensor(
                            out=wbfs[h], in0=wbfs[h],
                            in1=crep[:, h, :, :, :].rearrange(
                                "p t n b -> p (t n b)"),
                            op=OP.mult,
                        )
                        lvl(nc.vector, wbfs[h], SL, tmpfs[h], HSL, 8)
                    yfins = []
                    for h in range(NH):
                        lvl(nc.vector, tmpfs[h], HSL, wbfs[h], SL, 4)
                        lvl(nc.vector, wbfs[h], SL, tmpfs[h], HSL, 2)
                        yfin = scanp.tile([128, L, BH], f16, tag="yfin")
                        yfins.append(yfin)
                        nc.vector.tensor_tensor(
                            out=yfin[:].rearrange("p t b -> p (t b)"),
                            in0=AP(tensor=tmpfs[h].tensor,
                                   offset=tmpfs[h].offset,
                                   ap=[tmpfs[h].ap[0], [HSL, L], [1, BH]]),
                            in1=AP(tensor=tmpfs[h].tensor,
                                   offset=tmpfs[h].offset + BH,
                                   ap=[tmpfs[h].ap[0], [HSL, L], [1, BH]]),
                            op=OP.add,
                        )
                        if mt == 0:
                            dbg(f"yfin_{d}_h{h}", yfin[:])
                    # skip term: ygated slot = D*xc + yfin, then z-gate
                    og = ygated[:, mt, :]
                    for h in range(NH):
                        nc.vector.scalar_tensor_tensor(
                            out=AP(tensor=og.tensor, offset=og.offset + h * BH,
                                   ap=[og.ap[0], [128, L], [1, BH]]),
                            in0=AP(tensor=xc[:, mt, :].tensor,
                                   offset=xc[:, mt, :].offset + h * BH,
                                   ap=[og.ap[0], [128, L], [1, BH]]),
                            scalar=aux[d][:, mt, 6:7],
                            in1=yfins[h][:].rearrange("p t b -> p (t b)"),
                            op0=OP.mult, op1=OP.add,
                        )
                    nc.vector.tensor_tensor(out=og, in0=og, in1=sz_t[:],
                                            op=OP.mult)
                    if d == "f":
                        nc.sync.dma_start(ygz_d[d][mt, :, :], og)
                    if after_mt and mt in after_mt:
                        after_mt[mt]()
                dbg(f"ygated_{d}", ygated[:])

            def phase4_group(d, dir_i, ng, mg, from_dram):
                pso = psO.tile([128, 3, 512], f32, tag="psO")
                for kt in range(NT):
                    wo_t = wstream.tile([128, 3, 128], f16, tag="wo_t")
                    nc.sync.dma_start(
                        wo_t[:],
                        dram_ap(
                            ins[f"wout_{d}"],
                            kt * 128 * D + mg * 384,
                            [[D, 128], [128, 3], [1, 128]],
                        ),
                    )
                    if from_dram:
                        yg_t = wstream.tile([128, 384], f16, tag="yg_t")
                        ygv = ygz_d[d][:, :, :]
                        nc.sync.dma_start(
                            yg_t[:],
                            AP(tensor=ygv.tensor,
                               offset=ygv.offset + kt * 128 * TOK + ng * 384,
                               ap=[[TOK, 128], [1, 384]]),
                        )
                        rhs = yg_t[:]
                    else:
                        rhs = ygated[:, kt, ng * 384:(ng + 1) * 384]
                    for m in range(3):
                        nc.tensor.matmul(
                            pso[:, m, 0:384], wo_t[:, m, :], rhs,
                            start=(kt == 0), stop=(kt == NT - 1),
                        )
                for m in range(3):
                    mt_e = mg * 3 + m
                    o = yout[:, mt_e, ng * 384:(ng + 1) * 384]
                    if dir_i == 0:
                        nc.scalar.copy(o, pso[:, m, 0:384])
                    else:
                        nc.vector.tensor_tensor(
                            out=o, in0=o, in1=pso[:, m, 0:384], op=OP.add)

            # ---- pipelined emission order ----
            for mt in range(NT):
                phase1_mt("f", mt)
            phase2("f")
            ph1b = {mt: (lambda m=mt: phase1_mt("b", m)) for mt in range(NT)}
            phase3("f", after_mt=ph1b)
            phase2("b")
            ph4f = {
                2: lambda: phase4_group("f", 0, 0, 0, True),
                5: lambda: phase4_group("f", 0, 0, 1, True),
                8: lambda: phase4_group("f", 0, 1, 0, True),
                11: lambda: phase4_group("f", 0, 1, 1, True),
            }
            phase3("b", after_mt=ph4f)
            for ng in range(2):
                for mg in range(2):
                    phase4_group("b", 1, ng, mg, False)

            dbg("yout", yout[:])
            # ---------------- phase 5: residual + LN -> out --------------
            for tt in range(ET):
                xtok = tr1.tile([128, D], f32, tag="xtok")
                nc.sync.dma_start(xtok[:], x_d[tt * 128:(tt + 1) * 128, :])
                r_t = tr1.tile([128, D], f32, tag="r_t")
                for eg, cnt in ((0, 4), (4, 2)):
                    psh = psT.tile([128, 4, 128], f16, tag="pst")
                    for e in range(cnt):
                        nc.tensor.transpose(
                            psh[:, e, :],
                            yout[:, eg + e, tt * 128:(tt + 1) * 128], identh[:]
                        )
                    for e in range(cnt):
                        ec = eg + e
                        nc.vector.tensor_tensor(
                            out=r_t[:, ec * 128:(ec + 1) * 128],
                            in0=psh[:, e, :], in1=xtok[:, ec * 128:(ec + 1) * 128],
                            op=OP.add,
                        )
                stats = tr1.tile([128, 3, nc.vector.BN_STATS_DIM], f32, tag="stats")
                for sub in range(3):
                    nc.vector.bn_stats(
                        out=stats[:, sub, :], in_=r_t[:, sub * 256:(sub + 1) * 256]
                    )
                mv = tr1.tile([128, nc.vector.BN_AGGR_DIM], f32, tag="mv")
                nc.vector.bn_aggr(out=mv[:], in_=stats[:])
                rstd = tr1.tile([128, 1], f32, tag="rstd")
                nc.scalar.activation(
                    out=rstd[:], in_=mv[:, 1:2], func=AF.Sqrt, bias=eps_t[:],
                )
                nc.vector.reciprocal(out=rstd[:], in_=rstd[:])
                nc.vector.tensor_scalar(
                    out=r_t[:], in0=r_t[:], scalar1=mv[:, 0:1], scalar2=rstd[:],
                    op0=OP.subtract, op1=OP.mult,
                )
                nc.vector.tensor_tensor(out=r_t[:], in0=r_t[:], in1=g_rep[:], op=OP.mult)
                nc.vector.tensor_tensor(out=r_t[:], in0=r_t[:], in1=b_rep[:], op=OP.add)
                nc.sync.dma_start(out_d[tt * 128:(tt + 1) * 128, :], r_t[:])

    nc.compile()
    return nc


def _prep_inputs(inputs):
    f16 = np.float16
    shared = {}
    for d in ("f", "b"):
        in_proj = np.asarray(inputs[f"{d}_in"], np.float32)      # [3072, 768]
        shared[f"win_{d}"] = np.ascontiguousarray(in_proj[:DI].T).astype(f16)
        wz_T = in_proj[DI:].T                                    # [768, 1536]
        shared[f"wz_{d}"] = np.ascontiguousarray(
            wz_T.reshape(KT, 128, NT, 128).transpose(2, 1, 0, 3)
        ).astype(f16)
        xp_T = np.asarray(inputs[f"{d}_xp"], np.float32).T       # [1536, 80]
        shared[f"wxp_{d}"] = np.ascontiguousarray(
            xp_T.reshape(NT, 128, 80).transpose(1, 0, 2)
        ).astype(f16)
        shared[f"wdt_{d}"] = np.ascontiguousarray(
            np.asarray(inputs[f"{d}_dtw"], np.float32).T
        ).astype(f16)                                            # [48, 1536]
        shared[f"wout_{d}"] = np.ascontiguousarray(
            np.asarray(inputs[f"{d}_out"], np.float32).T
        ).astype(f16)                                            # [1536, 768]
        aux = np.zeros((DI, 8), np.float32)
        aux[:, 0:4] = np.asarray(inputs[f"{d}_cw"], np.float32).T
        aux[:, 4] = np.asarray(inputs[f"{d}_cb"], np.float32)
        aux[:, 5] = np.asarray(inputs[f"{d}_dtb"], np.float32)
        aux[:, 6] = np.asarray(inputs[f"{d}_D"], np.float32)
        shared[f"aux_{d}"] = aux
        cw = np.asarray(inputs[f"{d}_cw"], np.float32)           # [4, DI]
        cwd = np.zeros((NT, 128, 4, 128), np.float32)
        idx = np.arange(128)
        for mt in range(NT):
            for k in range(4):
                cwd[mt, idx, k, idx] = cw[k, mt * 128:(mt + 1) * 128]
        shared[f"cwd_{d}"] = cwd.astype(f16)
    shared["ln_g"] = np.ascontiguousarray(np.asarray(inputs["ln_g"], np.float32))
    shared["ln_b"] = np.ascontiguousarray(np.asarray(inputs["ln_b"], np.float32))
    return shared


def kernel(**inputs):
    from concourse import bass_utils

    if "nc" not in _CACHE:
        _CACHE["nc"] = _build_module()
    nc = _CACHE["nc"]

    shared = _prep_inputs(inputs)
    x = np.asarray(inputs["x"], np.float32)
    n_cores = 8
    bs = x.shape[0] // n_cores

    in_maps = []
    for c in range(n_cores):
        m = dict(shared)
        # l-major token order: row t*B + b
        m["x"] = np.ascontiguousarray(
            x[c * bs:(c + 1) * bs].transpose(1, 0, 2).reshape(TOK, D)
        ).astype(np.float32)
        in_maps.append(m)

    res = bass_utils.run_bass_kernel_spmd(nc, in_maps, core_ids=list(range(n_cores)))
    out = np.concatenate(
        [r["out"].reshape(L, bs, D).transpose(1, 0, 2) for r in res.results],
        axis=0,
    )
    return out.astype(np.float32)
